# revision 1
# baseline (speedup 1.0000x reference)
"""Masked multi-head SDP attention on 8 NeuronCores (head-parallel).

B=4, S=2048, D=1024, H=16, DK=64. Each core owns 2 heads: computes
Q/K/V projections for its heads (x replicated, DMA'd directly in
[d, s] transposed layout), causal flash-style attention with scores
kept transposed ([t, sq]) so the attn@V matmul needs no transposes,
softmax denominators fused into the V matmul via an appended ones
column, per-head normalization, and a partial output projection
through its 128 rows of W_O. Host sums the 8 partials and adds b_o.

Matmuls run as float32r (full PE rate at free-dim >= 256, ~1e-3 max
relative error vs fp32 -- measured on HW).
"""

import sys

sys.path.insert(0, "/opt/trn_rl_repo")

import numpy as np

import concourse.bass as bass
import concourse.mybir as mybir
from concourse import bacc
from concourse.masks import make_identity
from concourse.tile import TileContext
from concourse.bass_utils import run_bass_kernel_spmd

B, S, D, H = 4, 2048, 1024, 16
DK = D // H  # 64
NCORES = 8
HPC = H // NCORES  # 2 heads per core
KH = HPC * DK  # 128 = stacked head dim per core
NT = S // 128  # 16 t-tiles per batch
NI = S // 512  # 4 sq-blocks per batch
DC = D // 128  # 8 d-chunks

F32 = mybir.dt.float32
F32R = mybir.dt.float32r


def build_nc():
    nc = bacc.Bacc("TRN2", target_bir_lowering=False, debug=False,
                   num_devices=NCORES)
    x = nc.dram_tensor("x", [B, S, D], F32R, kind="ExternalInput").ap()
    wq = nc.dram_tensor("wq", [DC, 128, KH], F32R, kind="ExternalInput").ap()
    wk = nc.dram_tensor("wk", [DC, 128, KH], F32R, kind="ExternalInput").ap()
    wv = nc.dram_tensor("wv", [DC, 128, KH], F32R, kind="ExternalInput").ap()
    bq = nc.dram_tensor("bq", [KH, 1], F32, kind="ExternalInput").ap()
    bk = nc.dram_tensor("bk", [KH, 1], F32, kind="ExternalInput").ap()
    bv = nc.dram_tensor("bv", [KH, 1], F32, kind="ExternalInput").ap()
    wo = nc.dram_tensor("wo", [KH, D], F32R, kind="ExternalInput").ap()
    tri = nc.dram_tensor("tri", [128, 128], F32R, kind="ExternalInput").ap()
    out = nc.dram_tensor("out", [B, S, D], F32, kind="ExternalOutput").ap()

    with TileContext(nc) as tc:
        with (
            tc.tile_pool(name="const", bufs=1) as cpool,
            tc.tile_pool(name="wts", bufs=1) as wpool,
            tc.tile_pool(name="xt", bufs=2) as xpool,
            tc.tile_pool(name="seq", bufs=2) as qpool,
            tc.tile_pool(name="vn", bufs=2) as vpool,
            tc.tile_pool(name="attn", bufs=4) as apool,
            tc.tile_pool(name="fin", bufs=2) as fpool,
            tc.tile_pool(name="pacc", bufs=2, space="PSUM") as ps_acc,
            tc.tile_pool(name="psc", bufs=3, space="PSUM") as ps_sc,
            tc.tile_pool(name="pv", bufs=2, space="PSUM") as ps_v,
            tc.tile_pool(name="ptr", bufs=1, space="PSUM") as ps_tr,
        ):
            # persistent constants
            ident = cpool.tile([128, 128], F32, tag="ident")
            make_identity(nc, ident[:])
            ones_r = cpool.tile([128, 64], F32R, tag="ones")
            nc.gpsimd.memset(ones_r[:].bitcast(F32), 1.0)
            tri2_sb = cpool.tile([128, 256], F32R, tag="tri")
            nc.gpsimd.memset(tri2_sb[:, 0:128].bitcast(F32), 0.0)
            nc.sync.dma_start(out=tri2_sb[:, 128:256], in_=tri)
            w_sb = {}
            for nm, src in (("q", wq), ("k", wk), ("v", wv)):
                t = cpool.tile([128, DC * KH], F32R, tag="w" + nm)
                nc.sync.dma_start(
                    out=t[:].rearrange("p (c k) -> p c k", c=DC),
                    in_=src.rearrange("c p k -> p c k"))
                w_sb[nm] = t
            b_sb = {}
            for nm, src in (("q", bq), ("k", bk), ("v", bv)):
                t = cpool.tile([KH, 1], F32, tag="b" + nm)
                nc.sync.dma_start(out=t[:], in_=src)
                b_sb[nm] = t
            wo_sb = cpool.tile([KH, D], F32R, tag="wo")
            nc.sync.dma_start(out=wo_sb[:], in_=wo)

            for b in range(B):
                # ---------------- projections ----------------
                qt_sb = qpool.tile([128, S], F32R, tag="qt")
                kt_sb = qpool.tile([128, S], F32R, tag="kt")
                vn = [vpool.tile([128, 2 * DK + 2], F32R, tag=f"vn{j}",
                                 name=f"vn_{b}_{j}") for j in range(NT)]
                for st in range(NI):
                    xns = []
                    for ss in range(4):
                        xn = xpool.tile([128, D], F32, tag=f"xn{ss}", bufs=1,
                                        name=f"xn_{b}_{st}_{ss}")
                        nc.sync.dma_start(
                            out=xn[:],
                            in_=x[b, st * 512 + ss * 128:
                                  st * 512 + (ss + 1) * 128, :].bitcast(F32))
                        xns.append(xn)
                    xts = []
                    for dc in range(DC):
                        xt = xpool.tile([128, 512], F32R, tag=f"xt{dc}")
                        tps = ps_tr.tile([128, 512], F32, tag="tr",
                                         name=f"tps_{b}_{st}_{dc}")
                        for ss in range(4):
                            nc.tensor.transpose(
                                tps[:, ss * 128:(ss + 1) * 128],
                                xns[ss][:, dc * 128:(dc + 1) * 128],
                                ident[:])
                        nc.vector.tensor_copy(xt[:], tps[:])
                        xts.append(xt)
                    for nm, dst in (("q", qt_sb), ("k", kt_sb), ("v", None)):
                        acc = ps_acc.tile([128, 512], F32, tag="acc")
                        for dc in range(DC):
                            nc.tensor.matmul(
                                acc[:], w_sb[nm][:, dc * KH:(dc + 1) * KH],
                                xts[dc][:], start=(dc == 0), stop=(dc == DC - 1))
                        if nm != "v":
                            # psum -> sbuf with per-partition bias add
                            nc.vector.tensor_scalar_add(
                                dst[:, st * 512:(st + 1) * 512], acc[:],
                                b_sb[nm][:])
                        else:
                            vtt = fpool.tile([128, 512], F32, tag="vtt", bufs=1)
                            nc.vector.tensor_scalar_add(vtt[:], acc[:],
                                                        b_sb["v"][:])
                            for q in range(4):  # transpose to V natural
                                j = st * 4 + q
                                tp = ps_tr.tile([128, 128], F32, tag="tr")
                                nc.tensor.transpose(
                                    tp[:], vtt[:, q * 128:(q + 1) * 128],
                                    ident[:])
                                nc.vector.tensor_copy(vn[j][:, 0:DK],
                                                      tp[:, 0:DK])
                                nc.vector.tensor_copy(
                                    vn[j][:, DK + 1:2 * DK + 1],
                                    tp[:, DK:2 * DK])
                                nc.gpsimd.memset(vn[j][:, DK:DK + 1].bitcast(F32), 1.0)
                                nc.gpsimd.memset(
                                    vn[j][:, 2 * DK + 1:2 * DK + 2]
                                    .bitcast(F32), 1.0)

                # ---------------- attention ----------------
                catt = fpool.tile([128, S], F32R, tag="catt", bufs=1)
                for i in range(NI):
                    vps = [ps_v.tile([65, 512], F32, tag="vv",
                                     name=f"vp_{b}_{i}_{h}") for h in range(2)]
                    blocks = []
                    for j in range(4 * i + 4):
                        doff = 128 * (j - 4 * i) if j >= 4 * i else 0
                        off = min(doff, 256)  # N=128 fp32r runs at 1/4 rate
                        w = 512 - off
                        sq0 = i * 512 + off
                        ats = []
                        for h in range(2):
                            kslc = slice(h * DK, (h + 1) * DK)
                            sp = ps_sc.tile([128, 512], F32, tag="sc",
                                            name=f"sp_{b}_{i}_{j}_{h}")
                            nc.tensor.matmul(
                                sp[:, off:], kt_sb[kslc, j * 128:(j + 1) * 128],
                                qt_sb[kslc, sq0:i * 512 + 512],
                                start=True, stop=True)
                            at = apool.tile([128, w], F32R,
                                            tag=f"at{j}h{h}", bufs=1,
                                            name=f"at_{b}_{i}_{j}_{h}")
                            nc.scalar.activation(
                                at[:, 0:w], sp[:, off:],
                                mybir.ActivationFunctionType.Exp,
                                scale=float(1.0 / np.sqrt(DK)))
                            if j >= 4 * i:
                                mw = doff - off + 128
                                nc.vector.tensor_mul(
                                    at[:, 0:mw], at[:, 0:mw],
                                    tri2_sb[:, 256 - mw:256])
                            ats.append(at)
                        blocks.append((j, off, w, ats))
                    for j, off, w, ats in blocks:
                        for h in range(2):
                            nc.tensor.matmul(
                                vps[h][:, off:],
                                vn[j][:, h * (DK + 1):(h + 1) * (DK + 1)],
                                ats[h][:, 0:w],
                                start=(j == 0), stop=(j == 4 * i + 3))
                    # normalize: recip of denom row, broadcast via PE, multiply
                    dn = fpool.tile([65, 1024], F32R, tag="dn", bufs=1)
                    for h in range(2):
                        nc.vector.tensor_copy(dn[64:65, h * 512:(h + 1) * 512],
                                              vps[h][64:65, :])
                    with nc.allow_low_precision(reason="f32r == f32 bytes"):
                        nc.vector.reciprocal(dn[64:65, :], dn[64:65, :])
                    nat1 = fpool.tile([64, 512], F32R, tag="nat1")
                    for h in range(2):
                        bc = ps_sc.tile([64, 512], F32, tag="sc")
                        nc.tensor.matmul(bc[:], ones_r[64:65, :],
                                         dn[64:65, h * 512:(h + 1) * 512],
                                         start=True, stop=True)
                        bcs = fpool.tile([64, 512], F32R, tag="bcs",
                                         name=f"bcs_{b}_{i}_{h}")
                        nc.vector.tensor_copy(bcs[:], bc[:])
                        dst = (catt[0:64, i * 512:(i + 1) * 512] if h == 0
                               else nat1[:])
                        nc.vector.tensor_mul(dst, vps[h][0:64, :], bcs[:])
                    # cross-partition hop: head1 rows into catt[64:128]
                    nc.sync.dma_start(
                        out=catt[64:128, i * 512:(i + 1) * 512], in_=nat1[:])

                    # output projection for this i-block's 4 s-tiles
                    for st in range(4 * i, 4 * i + 4):
                        ob = fpool.tile([128, D], F32, tag="ob",
                                        name=f"ob_{b}_{st}")
                        for half in range(2):
                            pw = ps_acc.tile([128, 512], F32, tag="acc",
                                             name=f"pw_{b}_{st}_{half}")
                            nc.tensor.matmul(
                                pw[:], catt[:, st * 128:(st + 1) * 128],
                                wo_sb[:, half * 512:(half + 1) * 512],
                                start=True, stop=True)
                            nc.vector.tensor_copy(
                                ob[:, half * 512:(half + 1) * 512], pw[:])
                        nc.sync.dma_start(
                            out=out[b, st * 128:(st + 1) * 128, :], in_=ob[:])
    nc.finalize()
    return nc


_NC_CACHE = {}


def _get_nc():
    if "nc" not in _NC_CACHE:
        _NC_CACHE["nc"] = build_nc()
    return _NC_CACHE["nc"]


def kernel(x, Wq, bq, Wk, bk, Wv, bv, Wo, bo):
    x = np.ascontiguousarray(np.asarray(x, dtype=np.float32))
    tri = np.triu(np.ones((128, 128), dtype=np.float32))
    in_maps = []
    for c in range(NCORES):
        h0, h1 = 2 * c, 2 * c + 1
        m = {
            "x": x,
            "tri": tri,
            "wo": np.ascontiguousarray(Wo[c * KH:(c + 1) * KH]).astype(
                np.float32),
        }
        for nm, W, bb in (("q", Wq, bq), ("k", Wk, bk), ("v", Wv, bv)):
            Wc = np.concatenate([W[h0], W[h1]], axis=1).astype(np.float32)
            m["w" + nm] = np.ascontiguousarray(Wc.reshape(DC, 128, KH))
            m["b" + nm] = np.concatenate([bb[h0], bb[h1]]).astype(
                np.float32).reshape(KH, 1)
        in_maps.append(m)
    nc = _get_nc()
    res = run_bass_kernel_spmd(nc, in_maps, list(range(NCORES)))
    acc = np.zeros((B, S, D), dtype=np.float32)
    for c in range(NCORES):
        acc += res.results[c]["out"]
    return acc + np.asarray(bo, dtype=np.float32)[None, None, :]



# revision 11
# speedup vs baseline: 1.2552x; 1.2552x over previous
"""Masked multi-head SDP attention on 8 NeuronCores (head-parallel, bf16).

B=4, S=2048, D=1024, H=16, DK=64. Each core owns 2 heads. All matmuls in
bf16 (full PE rate at any free size; rel-err budget 2e-2 >> bf16 noise).

Key structure vs the fp32r baseline:
- x is cast to bf16 on host and loaded straight into [d, s] layout with the
  XBAR DMA-transpose (no PE transposes, no psum->sbuf copies for x^T).
- Scores for both heads of a j-tile go into one [128, 1024] PSUM pair and
  are exponentiated by a single 2-region Act instruction.
- V tiles are stored [v0 | v1] per t-tile plus one shared all-ones block;
  each head's attn@V matmul uses a 2-region lhsT [v_h | ones] so PSUM rows
  64:128 accumulate the softmax denominator replicated 64x. Normalization
  is then reciprocal + elementwise mul (cross-partition-base APs), with no
  PE broadcast, no denominator copies, and no cross-partition DMA hop.
- Output projection partials drain PSUM via the (otherwise idle) Pool
  engine and are written to DRAM as bf16; host sums the 8 partials in f32.
"""

import sys

sys.path.insert(0, "/opt/trn_rl_repo")

import numpy as np
import ml_dtypes

import concourse.bass as bass
import concourse.mybir as mybir
from concourse import bacc
from concourse.masks import make_identity
from concourse.tile import TileContext
from concourse.bass_utils import run_bass_kernel_spmd

B, S, D, H = 4, 2048, 1024, 16
DK = D // H  # 64
NCORES = 8
HPC = H // NCORES  # 2 heads per core
KH = HPC * DK  # 128
NT = S // 128  # 16 t-tiles per batch
NI = S // 512  # 4 sq-blocks per batch
DC = D // 128  # 8 d-chunks

F32 = mybir.dt.float32
BF16 = mybir.dt.bfloat16

VW = 3 * DK  # 192: [v0 | ones | v1] columns per t-tile in vna


def build_nc():
    nc = bacc.Bacc("TRN2", target_bir_lowering=False, debug=False,
                   num_devices=NCORES)
    x = nc.dram_tensor("x", [B, S, D], BF16, kind="ExternalInput").ap()
    wq = nc.dram_tensor("wq", [DC, 128, KH], BF16, kind="ExternalInput").ap()
    wk = nc.dram_tensor("wk", [DC, 128, KH], BF16, kind="ExternalInput").ap()
    wv = nc.dram_tensor("wv", [DC, 128, KH], BF16, kind="ExternalInput").ap()
    bq = nc.dram_tensor("bq", [KH, 1], F32, kind="ExternalInput").ap()
    bk = nc.dram_tensor("bk", [KH, 1], F32, kind="ExternalInput").ap()
    bv = nc.dram_tensor("bv", [KH, 1], F32, kind="ExternalInput").ap()
    wo = nc.dram_tensor("wo", [KH, D], BF16, kind="ExternalInput").ap()
    tri = nc.dram_tensor("tri", [128, 128], BF16, kind="ExternalInput").ap()
    out = nc.dram_tensor("out", [B, S, D], BF16, kind="ExternalOutput").ap()

    with TileContext(nc) as tc:
        with (
            tc.tile_pool(name="const", bufs=1) as cpool,
            tc.tile_pool(name="xt", bufs=2) as xpool,
            tc.tile_pool(name="seq", bufs=2) as qpool,
            tc.tile_pool(name="vn", bufs=2) as vpool,
            tc.tile_pool(name="attn", bufs=1) as apool,
            tc.tile_pool(name="fin", bufs=2) as fpool,
            tc.tile_pool(name="pacc", bufs=2, space="PSUM") as ps_acc,
            tc.tile_pool(name="psc", bufs=2, space="PSUM") as ps_sc,
            tc.tile_pool(name="pv", bufs=1, space="PSUM") as ps_v,
        ):
            # persistent constants
            ident = cpool.tile([128, 128], BF16, tag="ident")
            make_identity(nc, ident[:])
            tri2_sb = cpool.tile([128, 256], BF16, tag="tri")
            nc.sync.dma_start(out=tri2_sb[:, 0:128], in_=tri)
            nc.sync.dma_start(out=tri2_sb[:, 128:256], in_=tri)
            w_sb = {}
            for nm, src in (("q", wq), ("k", wk), ("v", wv)):
                t = cpool.tile([128, DC * KH], BF16, tag="w" + nm)
                nc.sync.dma_start(
                    out=t[:].rearrange("p (c k) -> p c k", c=DC),
                    in_=src.rearrange("c p k -> p c k"))
                w_sb[nm] = t
            b_sb = {}
            for nm, src in (("q", bq), ("k", bk), ("v", bv)):
                t = cpool.tile([KH, 1], F32, tag="b" + nm)
                nc.sync.dma_start(out=t[:], in_=src)
                b_sb[nm] = t
            wo_sb = cpool.tile([KH, D], BF16, tag="wo")
            nc.sync.dma_start(out=wo_sb[:], in_=wo)

            for b in range(B):
                # ---------------- x^T via XBAR dma transpose ----------------
                xts = []
                for dc in range(DC):
                    xt = xpool.tile([128, S], BF16, tag=f"xt{dc}")
                    for sh in range(2):  # split so proj can start early
                        nc.sync.dma_start(
                            out=xt[:, sh * 1024:(sh + 1) * 1024],
                            in_=x[b, sh * 1024:(sh + 1) * 1024,
                                  dc * 128:(dc + 1) * 128],
                            transpose=True)
                    xts.append(xt)

                # vna: per t-tile j, cols [j*192, (j+1)*192) = [v0 | ones | v1]
                # (V natural layout, bf16). head0 lhsT = cols [0:128) of the
                # block -> PSUM rows [v0out; den]; head1 lhsT = cols [64:192)
                # -> PSUM rows [den; v1out].
                vna = vpool.tile([128, NT * VW], BF16, tag="vna")
                vna_r = vna[:].rearrange("p (j g c) -> p j g c", j=NT, g=3)
                nc.gpsimd.memset(vna_r[:, :, 1:2, :], 1.0)
                qt_sb = qpool.tile([128, S], BF16, tag="qt")
                kt_sb = qpool.tile([128, S], BF16, tag="kt")

                # ---------------- projections ----------------
                for st in range(NI):
                    sl = slice(st * 512, (st + 1) * 512)
                    for nm, dst in (("q", qt_sb), ("k", kt_sb), ("v", None)):
                        acc = ps_acc.tile([128, 512], F32, tag="acc")
                        for dc in range(DC):
                            nc.tensor.matmul(
                                acc[:], w_sb[nm][:, dc * KH:(dc + 1) * KH],
                                xts[dc][:, sl], start=(dc == 0),
                                stop=(dc == DC - 1))
                        if nm != "v":
                            nc.vector.tensor_scalar_add(
                                dst[:, sl], acc[:], b_sb[nm][:])
                        else:
                            vtt = fpool.tile([128, 512], BF16, tag="vtt")
                            nc.vector.tensor_scalar_add(vtt[:], acc[:],
                                                        b_sb["v"][:])
                            tp = ps_acc.tile([128, 512], F32, tag="acc")
                            tpb = tp[:].bitcast(BF16)  # [128, 1024] bf16 view
                            for q in range(4):
                                nc.tensor.transpose(
                                    tpb[:, q * 128:(q + 1) * 128],
                                    vtt[:, q * 128:(q + 1) * 128], ident[:])
                            tp_r = tpb[:, 0:512].rearrange(
                                "p (q g c) -> p q g c", q=4, g=2)
                            js = slice(st * 4, (st + 1) * 4)
                            nc.vector.tensor_copy(
                                vna_r[:, js, 0:1, :], tp_r[:, :, 0:1, :])
                            nc.vector.tensor_copy(
                                vna_r[:, js, 2:3, :], tp_r[:, :, 1:2, :])

                # ---------------- attention ----------------
                catt = fpool.tile([128, S], BF16, tag="catt")
                for i in range(NI):
                    nj = 4 * i + 4
                    vps = [ps_v.tile([128, 512], F32, tag=f"v{h}",
                                     name=f"vp_{b}_{i}_{h}") for h in range(2)]
                    sq0 = i * 512
                    pend = []  # V-matmuls issued with lag 1
                    for j in range(nj):
                        q = j - 4 * i
                        off = 128 * q if q >= 0 else 0
                        w = 512 - off
                        sp = ps_sc.tile([128, 1024], F32, tag="sc",
                                        name=f"sp_{b}_{i}_{j}")
                        for h in range(2):
                            ks = slice(h * DK, (h + 1) * DK)
                            nc.tensor.matmul(
                                sp[:, h * 512 + off:h * 512 + 512],
                                kt_sb[ks, j * 128:(j + 1) * 128],
                                qt_sb[ks, sq0 + off:sq0 + 512],
                                start=True, stop=True)
                        at = apool.tile([128, 1024], BF16, tag=f"at{j}",
                                        name=f"at_{b}_{i}_{j}")
                        sp2 = sp[:].rearrange("p (r c) -> p r c", r=2)
                        at2 = at[:].rearrange("p (r c) -> p r c", r=2)
                        nc.scalar.activation(
                            at2[:, :, off:512], sp2[:, :, off:512],
                            mybir.ActivationFunctionType.Exp, scale=0.125)
                        if q >= 0:
                            nc.gpsimd.tensor_mul(
                                at2[:, :, off:off + 128],
                                at2[:, :, off:off + 128],
                                tri2_sb[:].rearrange("p (r c) -> p r c", r=2))
                        pend.append((j, off, at))
                        if len(pend) > 1:
                            self_flush(nc, vps, vna, pend.pop(0), nj)
                    while pend:
                        self_flush(nc, vps, vna, pend.pop(0), nj)

                    # normalize: rcp rows 0:64 <- 1/den(h0), 64:128 <- 1/den(h1)
                    rcp = fpool.tile([128, 512], F32, tag="rcp",
                                     name=f"rcp_{b}_{i}")
                    nc.vector.reciprocal(rcp[0:64, :], vps[0][64:128, :])
                    nc.vector.reciprocal(rcp[64:128, :], vps[1][0:64, :])
                    nc.vector.tensor_mul(catt[0:64, sq0:sq0 + 512],
                                         vps[0][0:64, :], rcp[0:64, :])
                    nc.vector.tensor_mul(catt[64:128, sq0:sq0 + 512],
                                         vps[1][64:128, :], rcp[64:128, :])

                    # output projection for this i-block's 4 s-tiles
                    for st in range(4 * i, 4 * i + 4):
                        pw = ps_sc.tile([128, 1024], F32, tag="sc",
                                        name=f"pw_{b}_{st}")
                        for half in range(2):
                            nc.tensor.matmul(
                                pw[:, half * 512:(half + 1) * 512],
                                catt[:, st * 128:(st + 1) * 128],
                                wo_sb[:, half * 512:(half + 1) * 512],
                                start=True, stop=True)
                        ob = fpool.tile([128, D], BF16, tag="ob",
                                        name=f"ob_{b}_{st}")
                        nc.vector.tensor_copy(ob[:], pw[:])
                        nc.sync.dma_start(
                            out=out[b, st * 128:(st + 1) * 128, :], in_=ob[:])
    nc.finalize()
    return nc


def self_flush(nc, vps, vna, item, nj):
    """Issue the two attn@V matmuls for a pending j-tile.

    head0 lhsT = [v0 | ones] -> vps[0] rows 0:64 = v@attn, 64:128 = denom
    head1 lhsT = [ones | v1] -> vps[1] rows 0:64 = denom, 64:128 = v@attn
    """
    j, off, at = item
    for h in range(2):
        base = j * VW + h * DK
        nc.tensor.matmul(
            vps[h][:, off:512], vna[:, base:base + 2 * DK],
            at[:, h * 512 + off:h * 512 + 512],
            start=(j == 0), stop=(j == nj - 1))


_NC_CACHE = {}


def _get_nc():
    if "nc" not in _NC_CACHE:
        _NC_CACHE["nc"] = build_nc()
    return _NC_CACHE["nc"]


def kernel(x, Wq, bq, Wk, bk, Wv, bv, Wo, bo):
    x_bf = np.ascontiguousarray(np.asarray(x, dtype=np.float32)).astype(
        ml_dtypes.bfloat16)
    tri = np.triu(np.ones((128, 128), dtype=np.float32)).astype(
        ml_dtypes.bfloat16)
    in_maps = []
    for c in range(NCORES):
        h0, h1 = 2 * c, 2 * c + 1
        m = {
            "x": x_bf,
            "tri": tri,
            "wo": np.ascontiguousarray(Wo[c * KH:(c + 1) * KH]).astype(
                ml_dtypes.bfloat16),
        }
        for nm, W, bb in (("q", Wq, bq), ("k", Wk, bk), ("v", Wv, bv)):
            Wc = np.concatenate([W[h0], W[h1]], axis=1).astype(
                ml_dtypes.bfloat16)
            m["w" + nm] = np.ascontiguousarray(Wc.reshape(DC, 128, KH))
            m["b" + nm] = np.concatenate([bb[h0], bb[h1]]).astype(
                np.float32).reshape(KH, 1)
        in_maps.append(m)
    nc = _get_nc()
    res = run_bass_kernel_spmd(nc, in_maps, list(range(NCORES)))
    acc = np.zeros((B, S, D), dtype=np.float32)
    for c in range(NCORES):
        acc += np.asarray(res.results[c]["out"], dtype=np.float32)
    return acc + np.asarray(bo, dtype=np.float32)[None, None, :]


# revision 14
# speedup vs baseline: 1.5271x; 1.2166x over previous
"""Masked multi-head SDP attention on 8 NeuronCores (head-parallel, bf16).

B=4, S=2048, D=1024, H=16, DK=64. Each core owns 2 heads. All matmuls in
bf16 (full PE rate at any free size; rel-err budget 2e-2 >> bf16 noise).

Structure:
- x cast to bf16 on host, loaded straight into [d, s] layout via the XBAR
  DMA-transpose (no PE transposes / psum copies for x^T).
- Scores for both heads of a j-tile go into one [128, 1024] PSUM pair,
  exponentiated by a single 2-region Act instruction; causal masking via a
  Pool-engine multiply with a duplicated triu tile.
- V tiles stored per t-tile as [v0 | ones | v1]; each head's attn@V lhsT is
  a contiguous 128-col window so PSUM rows accumulate both v@attn and the
  softmax denominator (replicated 64x). Normalization = reciprocal + mul
  with cross-partition-base APs; no PE broadcast, no DMA hop.
- Software pipelining: the projection work for batch b+1 is emitted in
  small chunks interleaved into batch b's attention j-stream (PE executes
  in program order, so fillers must be interleaved at emission time), and
  each i-block's output projection is pushed as filler into the following
  stream. The serial DVE normalize chain is covered by filler matmuls.
- Output-projection partials drain PSUM via DVE as bf16 and are written to
  DRAM as bf16; host sums the 8 partials in f32.
"""

import sys

sys.path.insert(0, "/opt/trn_rl_repo")

import collections
import numpy as np
import ml_dtypes

import concourse.bass as bass
import concourse.mybir as mybir
from concourse import bacc
from concourse.masks import make_identity
from concourse.tile import TileContext
from concourse.bass_utils import run_bass_kernel_spmd

B, S, D, H = 4, 2048, 1024, 16
DK = D // H  # 64
NCORES = 8
HPC = H // NCORES  # 2 heads per core
KH = HPC * DK  # 128
NT = S // 128  # 16 t-tiles per batch
NI = S // 512  # 4 sq-blocks per batch
DC = D // 128  # 8 d-chunks

F32 = mybir.dt.float32
BF16 = mybir.dt.bfloat16

VW = 3 * DK  # 192: [v0 | ones | v1] columns per t-tile in vna


def build_nc():
    nc = bacc.Bacc("TRN2", target_bir_lowering=False, debug=False,
                   num_devices=NCORES)
    x = nc.dram_tensor("x", [B, S, D], BF16, kind="ExternalInput").ap()
    wq = nc.dram_tensor("wq", [DC, 128, KH], BF16, kind="ExternalInput").ap()
    wk = nc.dram_tensor("wk", [DC, 128, KH], BF16, kind="ExternalInput").ap()
    wv = nc.dram_tensor("wv", [DC, 128, KH], BF16, kind="ExternalInput").ap()
    bq = nc.dram_tensor("bq", [KH, 1], F32, kind="ExternalInput").ap()
    bk = nc.dram_tensor("bk", [KH, 1], F32, kind="ExternalInput").ap()
    bv = nc.dram_tensor("bv", [KH, 1], F32, kind="ExternalInput").ap()
    wo = nc.dram_tensor("wo", [KH, D], BF16, kind="ExternalInput").ap()
    tri = nc.dram_tensor("tri", [128, 128], BF16, kind="ExternalInput").ap()
    out = nc.dram_tensor("out", [B, S, D], BF16, kind="ExternalOutput").ap()

    with TileContext(nc) as tc:
        with (
            tc.tile_pool(name="const", bufs=1) as cpool,
            tc.tile_pool(name="xt", bufs=2) as xpool,
            tc.tile_pool(name="seq", bufs=2) as qpool,
            tc.tile_pool(name="vn", bufs=2) as vpool,
            tc.tile_pool(name="attn", bufs=1) as apool,
            tc.tile_pool(name="fin", bufs=2) as fpool,
            tc.tile_pool(name="pacc", bufs=2, space="PSUM") as ps_acc,
            tc.tile_pool(name="psc", bufs=2, space="PSUM") as ps_sc,
            tc.tile_pool(name="pv", bufs=1, space="PSUM") as ps_v,
        ):
            # weights first (needed by the first projection matmuls)
            w_sb = {}
            for nm, src in (("q", wq), ("k", wk), ("v", wv)):
                t = cpool.tile([128, DC * KH], BF16, tag="w" + nm,
                               name=f"w_{nm}")
                nc.sync.dma_start(
                    out=t[:].rearrange("p (c k) -> p c k", c=DC),
                    in_=src.rearrange("c p k -> p c k"))
                w_sb[nm] = t
            b_sb = {}
            for nm, src in (("q", bq), ("k", bk), ("v", bv)):
                t = cpool.tile([KH, 1], F32, tag="b" + nm, name=f"b_{nm}")
                nc.sync.dma_start(out=t[:], in_=src)
                b_sb[nm] = t

            def prepare(b, splits=2):
                """Allocate per-batch tiles and issue x^T XBAR DMAs."""
                xts = []
                sw = S // splits
                for dc in range(DC):
                    xt = xpool.tile([128, S], BF16, tag=f"xt{dc}",
                                    name=f"xt_{b}_{dc}")
                    for sh in range(splits):
                        nc.sync.dma_start(
                            out=xt[:, sh * sw:(sh + 1) * sw],
                            in_=x[b, sh * sw:(sh + 1) * sw,
                                  dc * 128:(dc + 1) * 128],
                            transpose=True)
                    xts.append(xt)
                vna = vpool.tile([128, NT * VW], BF16, tag="vna",
                                 name=f"vna_{b}")
                vna_r = vna[:].rearrange("p (j g c) -> p j g c", j=NT, g=3)
                nc.gpsimd.memset(vna_r[:, :, 1:2, :], 1.0)
                qt = qpool.tile([128, S], BF16, tag="qt", name=f"qt_{b}")
                kt = qpool.tile([128, S], BF16, tag="kt", name=f"kt_{b}")
                return dict(b=b, xts=xts, vna=vna, vna_r=vna_r, qt=qt, kt=kt)

            ident = cpool.tile([128, 128], BF16, tag="ident")
            make_identity(nc, ident[:])
            tri2_sb = cpool.tile([128, 256], BF16, tag="tri")
            nc.sync.dma_start(out=tri2_sb[:, 0:128], in_=tri)
            nc.sync.dma_start(out=tri2_sb[:, 128:256], in_=tri)
            wo_sb = cpool.tile([KH, D], BF16, tag="wo")
            nc.sync.dma_start(out=wo_sb[:], in_=wo)

            def proj_gen(ctx):
                """Generator emitting batch ctx's q/k/v projections in small
                PE chunks (yield points let attention interleave)."""
                b = ctx["b"]
                for st in range(NI):
                    sl = slice(st * 512, (st + 1) * 512)
                    for nm in ("q", "k", "v"):
                        acc = ps_acc.tile([128, 512], F32, tag="acc",
                                          name=f"acc_{b}_{st}_{nm}")
                        for dc0 in range(0, DC, 2):
                            for dc in (dc0, dc0 + 1):
                                nc.tensor.matmul(
                                    acc[:],
                                    w_sb[nm][:, dc * KH:(dc + 1) * KH],
                                    ctx["xts"][dc][:, sl], start=(dc == 0),
                                    stop=(dc == DC - 1))
                            yield
                        if nm != "v":
                            dst = ctx["qt"] if nm == "q" else ctx["kt"]
                            nc.vector.tensor_scalar_add(
                                dst[:, sl], acc[:], b_sb[nm][:])
                        else:
                            vtt = fpool.tile([128, 512], BF16, tag="vtt",
                                             name=f"vtt_{b}_{st}")
                            nc.vector.tensor_scalar_add(vtt[:], acc[:],
                                                        b_sb["v"][:])
                            tp = ps_acc.tile([128, 512], F32, tag="acc",
                                             name=f"tp_{b}_{st}")
                            tpb = tp[:].bitcast(BF16)
                            for q in range(4):
                                nc.tensor.transpose(
                                    tpb[:, q * 128:(q + 1) * 128],
                                    vtt[:, q * 128:(q + 1) * 128], ident[:])
                            yield
                            tp_r = tpb[:, 0:512].rearrange(
                                "p (q g c) -> p q g c", q=4, g=2)
                            js = slice(st * 4, (st + 1) * 4)
                            nc.vector.tensor_copy(
                                ctx["vna_r"][:, js, 0:1, :], tp_r[:, :, 0:1, :])
                            nc.vector.tensor_copy(
                                ctx["vna_r"][:, js, 2:3, :], tp_r[:, :, 1:2, :])
                            yield

            oneshot = collections.deque()
            gen_box = [None]

            def pull(n):
                for _ in range(n):
                    if gen_box[0] is not None:
                        try:
                            next(gen_box[0])
                            continue
                        except StopIteration:
                            gen_box[0] = None
                    if oneshot:
                        oneshot.popleft()()

            def drain_all():
                while oneshot or gen_box[0] is not None:
                    pull(1)

            def flush_v(vps, vna, item, nj):
                """attn@V for a pending j-tile. head0 lhsT [v0|ones] ->
                rows 0:64 v@attn, 64:128 denom; head1 lhsT [ones|v1] ->
                rows 0:64 denom, 64:128 v@attn."""
                j, off, at = item
                for h in range(2):
                    base = j * VW + h * DK
                    nc.tensor.matmul(
                        vps[h][:, off:512], vna[:, base:base + 2 * DK],
                        at[:, h * 512 + off:h * 512 + 512],
                        start=(j == 0), stop=(j == nj - 1))

            def outproj_tile(b, st, catt):
                def emit():
                    ob = fpool.tile([128, D], BF16, tag="ob",
                                    name=f"ob_{b}_{st}")
                    for half in range(2):
                        pw = ps_acc.tile([128, 512], F32, tag="acc",
                                         name=f"pw_{b}_{st}_{half}")
                        nc.tensor.matmul(
                            pw[:], catt[:, st * 128:(st + 1) * 128],
                            wo_sb[:, half * 512:(half + 1) * 512],
                            start=True, stop=True)
                        nc.vector.tensor_copy(
                            ob[:, half * 512:(half + 1) * 512], pw[:])
                    nc.sync.dma_start(
                        out=out[b, st * 128:(st + 1) * 128, :], in_=ob[:])
                return emit

            def attention(ctx):
                b = ctx["b"]
                qt, kt, vna = ctx["qt"], ctx["kt"], ctx["vna"]
                catt = fpool.tile([128, S], BF16, tag="catt",
                                  name=f"catt_{b}")
                for i in range(NI):
                    nj = 4 * i + 4
                    vps = [ps_v.tile([128, 512], F32, tag=f"v{h}",
                                     name=f"vp_{b}_{i}_{h}")
                           for h in range(2)]
                    sq0 = i * 512
                    pend = []
                    for j in range(nj):
                        q = j - 4 * i
                        off = 128 * q if q >= 0 else 0
                        sp = ps_sc.tile([128, 1024], F32, tag="sc",
                                        name=f"sp_{b}_{i}_{j}")
                        for h in range(2):
                            ks = slice(h * DK, (h + 1) * DK)
                            nc.tensor.matmul(
                                sp[:, h * 512 + off:h * 512 + 512],
                                kt[ks, j * 128:(j + 1) * 128],
                                qt[ks, sq0 + off:sq0 + 512],
                                start=True, stop=True)
                        at = apool.tile([128, 1024], BF16, tag=f"at{j}",
                                        name=f"at_{b}_{i}_{j}")
                        sp2 = sp[:].rearrange("p (r c) -> p r c", r=2)
                        at2 = at[:].rearrange("p (r c) -> p r c", r=2)
                        nc.scalar.activation(
                            at2[:, :, off:512], sp2[:, :, off:512],
                            mybir.ActivationFunctionType.Exp, scale=0.125)
                        if q >= 0:
                            nc.gpsimd.tensor_mul(
                                at2[:, :, off:off + 128],
                                at2[:, :, off:off + 128],
                                tri2_sb[:].rearrange("p (r c) -> p r c", r=2))
                        pend.append((j, off, at))
                        if len(pend) > 1:
                            flush_v(vps, vna, pend.pop(0), nj)
                        pull(1)
                    while pend:
                        flush_v(vps, vna, pend.pop(0), nj)

                    # normalize (DVE chain, covered by pulled PE fillers)
                    rcp = fpool.tile([128, 512], F32, tag="rcp",
                                     name=f"rcp_{b}_{i}")
                    nc.vector.reciprocal(rcp[0:64, :], vps[0][64:128, :])
                    nc.vector.reciprocal(rcp[64:128, :], vps[1][0:64, :])
                    nc.vector.tensor_mul(catt[0:64, sq0:sq0 + 512],
                                         vps[0][0:64, :], rcp[0:64, :])
                    nc.vector.tensor_mul(catt[64:128, sq0:sq0 + 512],
                                         vps[1][64:128, :], rcp[64:128, :])
                    for st in range(4 * i, 4 * i + 4):
                        oneshot.append(outproj_tile(b, st, catt))
                    pull(3)

            # batch 0's projections run up front (nothing to hide behind)
            ctx = prepare(0)
            gen_box[0] = proj_gen(ctx)
            drain_all()
            for b in range(B):
                nxt = prepare(b + 1) if b + 1 < B else None
                gen_box[0] = proj_gen(nxt) if nxt else None
                attention(ctx)
                drain_all()
                ctx = nxt
    nc.finalize()
    return nc


_NC_CACHE = {}


def _get_nc():
    if "nc" not in _NC_CACHE:
        _NC_CACHE["nc"] = build_nc()
    return _NC_CACHE["nc"]


def kernel(x, Wq, bq, Wk, bk, Wv, bv, Wo, bo):
    x_bf = np.ascontiguousarray(np.asarray(x, dtype=np.float32)).astype(
        ml_dtypes.bfloat16)
    tri = np.triu(np.ones((128, 128), dtype=np.float32)).astype(
        ml_dtypes.bfloat16)
    in_maps = []
    for c in range(NCORES):
        h0, h1 = 2 * c, 2 * c + 1
        m = {
            "x": x_bf,
            "tri": tri,
            "wo": np.ascontiguousarray(Wo[c * KH:(c + 1) * KH]).astype(
                ml_dtypes.bfloat16),
        }
        for nm, W, bb in (("q", Wq, bq), ("k", Wk, bk), ("v", Wv, bv)):
            Wc = np.concatenate([W[h0], W[h1]], axis=1).astype(
                ml_dtypes.bfloat16)
            m["w" + nm] = np.ascontiguousarray(Wc.reshape(DC, 128, KH))
            m["b" + nm] = np.concatenate([bb[h0], bb[h1]]).astype(
                np.float32).reshape(KH, 1)
        in_maps.append(m)
    nc = _get_nc()
    res = run_bass_kernel_spmd(nc, in_maps, list(range(NCORES)))
    acc = np.zeros((B, S, D), dtype=np.float32)
    for c in range(NCORES):
        acc += np.asarray(res.results[c]["out"], dtype=np.float32)
    return acc + np.asarray(bo, dtype=np.float32)[None, None, :]


# revision 41
# speedup vs baseline: 1.5693x; 1.0276x over previous
"""Masked multi-head SDP attention on 8 NeuronCores (head-parallel, bf16).

B=4, S=2048, D=1024, H=16, DK=64. Each core owns 2 heads. All matmuls in
bf16 (full PE rate at any free size; rel-err budget 2e-2 >> bf16 noise).

Structure:
- x cast to bf16 on host, loaded straight into [d, s] layout via the XBAR
  DMA-transpose (no PE transposes / psum copies for x^T).
- Scores for both heads of a j-tile go into one [128, 1024] PSUM pair,
  exponentiated by a single 2-region Act instruction; causal masking via a
  Pool-engine multiply with a duplicated triu tile.
- V tiles stored per t-tile as [v0 | ones | v1]; each head's attn@V lhsT is
  a contiguous 128-col window so PSUM rows accumulate both v@attn and the
  softmax denominator (replicated 64x). Normalization = reciprocal + mul
  with cross-partition-base APs; no PE broadcast, no DMA hop.
- Software pipelining: the projection work for batch b+1 is emitted in
  small chunks interleaved into batch b's attention j-stream (PE executes
  in program order, so fillers must be interleaved at emission time), and
  each i-block's output projection is pushed as filler into the following
  stream. The serial DVE normalize chain is covered by filler matmuls.
- Output-projection partials drain PSUM via DVE as bf16 and are written to
  DRAM as bf16; host sums the 8 partials in f32.
"""

import sys

sys.path.insert(0, "/opt/trn_rl_repo")

import collections
import numpy as np
import ml_dtypes

import concourse.bass as bass
import concourse.mybir as mybir
from concourse import bacc
from concourse.masks import make_identity
from concourse.tile import TileContext
from concourse.bass_utils import run_bass_kernel_spmd

B, S, D, H = 4, 2048, 1024, 16
DK = D // H  # 64
NCORES = 8
HPC = H // NCORES  # 2 heads per core
KH = HPC * DK  # 128
NT = S // 128  # 16 t-tiles per batch
NI = S // 512  # 4 sq-blocks per batch
DC = D // 128  # 8 d-chunks

F32 = mybir.dt.float32
BF16 = mybir.dt.bfloat16

VW = 3 * DK  # 192: [v0 | ones | v1] columns per t-tile in vna
# filler pulls per j by i-block: early blocks skip pulls (the next batch's
# x^T DMAs haven't landed yet), later blocks pull harder
PULLS = [1, 1, 1, 1]


def build_nc():
    nc = bacc.Bacc("TRN2", target_bir_lowering=False, debug=False,
                   num_devices=NCORES)
    x = nc.dram_tensor("x", [B, S, D], BF16, kind="ExternalInput").ap()
    wq = nc.dram_tensor("wq", [DC, 128, KH], BF16, kind="ExternalInput").ap()
    wk = nc.dram_tensor("wk", [DC, 128, KH], BF16, kind="ExternalInput").ap()
    wv = nc.dram_tensor("wv", [DC, 128, KH], BF16, kind="ExternalInput").ap()
    bq = nc.dram_tensor("bq", [KH, 1], F32, kind="ExternalInput").ap()
    bk = nc.dram_tensor("bk", [KH, 1], F32, kind="ExternalInput").ap()
    bv = nc.dram_tensor("bv", [KH, 1], F32, kind="ExternalInput").ap()
    wo = nc.dram_tensor("wo", [KH, D], BF16, kind="ExternalInput").ap()
    tri = nc.dram_tensor("tri", [128, 128], BF16, kind="ExternalInput").ap()
    out = nc.dram_tensor("out", [B, S, D], BF16, kind="ExternalOutput").ap()

    with TileContext(nc) as tc:
        with (
            tc.tile_pool(name="const", bufs=1) as cpool,
            tc.tile_pool(name="xt", bufs=2) as xpool,
            tc.tile_pool(name="seq", bufs=2) as qpool,
            tc.tile_pool(name="vn", bufs=2) as vpool,
            tc.tile_pool(name="attn", bufs=1) as apool,
            tc.tile_pool(name="fin", bufs=2) as fpool,
            tc.tile_pool(name="pacc", bufs=2, space="PSUM") as ps_acc,
            tc.tile_pool(name="psc", bufs=2, space="PSUM") as ps_sc,
            tc.tile_pool(name="pv", bufs=1, space="PSUM") as ps_v,
        ):
            # weights first (needed by the first projection matmuls)
            w_sb = {}
            for nm, src in (("q", wq), ("k", wk), ("v", wv)):
                t = cpool.tile([128, DC * KH], BF16, tag="w" + nm,
                               name=f"w_{nm}")
                nc.sync.dma_start(
                    out=t[:].rearrange("p (c k) -> p c k", c=DC),
                    in_=src.rearrange("c p k -> p c k"))
                w_sb[nm] = t
            b_sb = {}
            for nm, src in (("q", bq), ("k", bk), ("v", bv)):
                t = cpool.tile([KH, 1], F32, tag="b" + nm, name=f"b_{nm}")
                nc.sync.dma_start(out=t[:], in_=src)
                b_sb[nm] = t

            def prepare(b, splits=2):
                """Allocate per-batch tiles and issue x^T XBAR DMAs."""
                sw = S // splits
                xts = [xpool.tile([128, S], BF16, tag=f"xt{dc}",
                                  name=f"xt_{b}_{dc}") for dc in range(DC)]
                for sh in range(splits):  # slice-major: st-block 0 lands first
                    for dc in range(DC):
                        nc.sync.dma_start(
                            out=xts[dc][:, sh * sw:(sh + 1) * sw],
                            in_=x[b, sh * sw:(sh + 1) * sw,
                                  dc * 128:(dc + 1) * 128],
                            transpose=True)
                vna = vpool.tile([128, NT * VW], BF16, tag="vna",
                                 name=f"vna_{b}")
                vna_r = vna[:].rearrange("p (j g c) -> p j g c", j=NT, g=3)
                nc.gpsimd.memset(vna_r[:, :, 1:2, :], 1.0)
                qt = qpool.tile([128, S], BF16, tag="qt", name=f"qt_{b}")
                kt = qpool.tile([128, S], BF16, tag="kt", name=f"kt_{b}")
                return dict(b=b, xts=xts, vna=vna, vna_r=vna_r, qt=qt, kt=kt)

            ident = cpool.tile([128, 128], BF16, tag="ident")
            make_identity(nc, ident[:])
            tri2_sb = cpool.tile([128, 256], BF16, tag="tri")
            nc.sync.dma_start(out=tri2_sb[:, 0:128], in_=tri)
            nc.sync.dma_start(out=tri2_sb[:, 128:256], in_=tri)
            wo_sb = cpool.tile([KH, D], BF16, tag="wo")
            nc.sync.dma_start(out=wo_sb[:], in_=wo)

            def proj_gen(ctx):
                """Generator emitting batch ctx's q/k/v projections in small
                PE chunks (yield points let attention interleave)."""
                b = ctx["b"]
                for st in range(NI):
                    sl = slice(st * 512, (st + 1) * 512)
                    for nm in ("q", "k", "v"):
                        acc = ps_acc.tile([128, 512], F32, tag="acc",
                                          name=f"acc_{b}_{st}_{nm}")
                        for dc0 in range(0, DC, 2):
                            for dc in (dc0, dc0 + 1):
                                nc.tensor.matmul(
                                    acc[:],
                                    w_sb[nm][:, dc * KH:(dc + 1) * KH],
                                    ctx["xts"][dc][:, sl], start=(dc == 0),
                                    stop=(dc == DC - 1))
                            yield
                        if nm != "v":
                            dst = ctx["qt"] if nm == "q" else ctx["kt"]
                            nc.vector.tensor_scalar_add(
                                dst[:, sl], acc[:], b_sb[nm][:])
                        else:
                            vtt = fpool.tile([128, 512], BF16, tag="vtt",
                                             name=f"vtt_{b}_{st}")
                            nc.vector.tensor_scalar_add(vtt[:], acc[:],
                                                        b_sb["v"][:])
                            tp = ps_acc.tile([128, 512], F32, tag="acc",
                                             name=f"tp_{b}_{st}")
                            tpb = tp[:].bitcast(BF16)
                            for q in range(4):
                                nc.tensor.transpose(
                                    tpb[:, q * 128:(q + 1) * 128],
                                    vtt[:, q * 128:(q + 1) * 128], ident[:])
                            yield
                            tp_r = tpb[:, 0:512].rearrange(
                                "p (q g c) -> p q g c", q=4, g=2)
                            js = slice(st * 4, (st + 1) * 4)
                            nc.vector.tensor_copy(
                                ctx["vna_r"][:, js, 0:1, :], tp_r[:, :, 0:1, :])
                            nc.vector.tensor_copy(
                                ctx["vna_r"][:, js, 2:3, :], tp_r[:, :, 1:2, :])
                            yield

            oneshot = collections.deque()
            gen_box = [None]

            def pull_gen(n=1):
                for _ in range(n):
                    if gen_box[0] is not None:
                        try:
                            next(gen_box[0])
                            continue
                        except StopIteration:
                            gen_box[0] = None
                    if oneshot:
                        oneshot.popleft()()

            def pull_one(n=1):
                for _ in range(n):
                    if oneshot:
                        oneshot.popleft()()
                    elif gen_box[0] is not None:
                        try:
                            next(gen_box[0])
                        except StopIteration:
                            gen_box[0] = None

            def drain_all():
                while oneshot or gen_box[0] is not None:
                    pull_gen(1)

            def flush_v(vps, vna, item, nj):
                """attn@V for a pending j-tile. head0 lhsT [v0|ones] ->
                rows 0:64 v@attn, 64:128 denom; head1 lhsT [ones|v1] ->
                rows 0:64 denom, 64:128 v@attn."""
                j, off, at = item
                for h in range(2):
                    base = j * VW + h * DK
                    nc.tensor.matmul(
                        vps[h][:, off:512], vna[:, base:base + 2 * DK],
                        at[:, h * 512 + off:h * 512 + 512],
                        start=(j == 0), stop=(j == nj - 1))

            def outproj_half(b, st, catt, ob, half, act_copy=False):
                def emit():
                    pw = ps_acc.tile([128, 512], F32, tag="acc",
                                     name=f"pw_{b}_{st}_{half}")
                    nc.tensor.matmul(
                        pw[:], catt[:, st * 128:(st + 1) * 128],
                        wo_sb[:, half * 512:(half + 1) * 512],
                        start=True, stop=True)
                    dst = ob[:, half * 512:(half + 1) * 512]
                    if act_copy:
                        nc.scalar.copy(dst, pw[:])
                    else:
                        nc.vector.tensor_copy(dst, pw[:])
                    if half == 1:
                        nc.sync.dma_start(
                            out=out[b, st * 128:(st + 1) * 128, :], in_=ob[:])
                return emit

            def attention(ctx):
                b = ctx["b"]
                qt, kt, vna = ctx["qt"], ctx["kt"], ctx["vna"]
                catt = fpool.tile([128, S], BF16, tag="catt",
                                  name=f"catt_{b}")
                for i in range(NI):
                    nj = 4 * i + 4
                    vps = [ps_v.tile([128, 512], F32, tag=f"v{h}",
                                     name=f"vp_{b}_{i}_{h}")
                           for h in range(2)]
                    sq0 = i * 512
                    pend = []
                    for j in range(nj):
                        q = j - 4 * i
                        off = 128 * q if q >= 0 else 0
                        sp = ps_sc.tile([128, 1024], F32, tag="sc",
                                        name=f"sp_{b}_{i}_{j}")
                        for h in range(2):
                            ks = slice(h * DK, (h + 1) * DK)
                            nc.tensor.matmul(
                                sp[:, h * 512 + off:h * 512 + 512],
                                kt[ks, j * 128:(j + 1) * 128],
                                qt[ks, sq0 + off:sq0 + 512],
                                start=True, stop=True)
                        at = apool.tile([128, 1024], BF16, tag=f"at{j}",
                                        name=f"at_{b}_{i}_{j}")
                        sp2 = sp[:].rearrange("p (r c) -> p r c", r=2)
                        at2 = at[:].rearrange("p (r c) -> p r c", r=2)
                        nc.scalar.activation(
                            at2[:, :, off:512], sp2[:, :, off:512],
                            mybir.ActivationFunctionType.Exp, scale=0.125)
                        if q >= 0:
                            nc.gpsimd.tensor_mul(
                                at2[:, :, off:off + 128],
                                at2[:, :, off:off + 128],
                                tri2_sb[:].rearrange("p (r c) -> p r c", r=2))
                        pend.append((j, off, at))
                        if len(pend) > 1:
                            flush_v(vps, vna, pend.pop(0), nj)
                        pull_gen(1)
                    while pend:
                        flush_v(vps, vna, pend.pop(0), nj)

                    # normalize (DVE chain, covered by pulled PE fillers)
                    rcp = fpool.tile([128, 512], F32, tag="rcp",
                                     name=f"rcp_{b}_{i}")
                    nc.vector.reciprocal(rcp[0:64, :], vps[0][64:128, :])
                    nc.vector.reciprocal(rcp[64:128, :], vps[1][0:64, :])
                    nc.vector.tensor_mul(catt[0:64, sq0:sq0 + 512],
                                         vps[0][0:64, :], rcp[0:64, :])
                    nc.vector.tensor_mul(catt[64:128, sq0:sq0 + 512],
                                         vps[1][64:128, :], rcp[64:128, :])
                    tail = b == B - 1 and i == NI - 1
                    for st in range(4 * i, 4 * i + 4):
                        ob = fpool.tile([128, D], BF16, tag="ob",
                                        name=f"ob_{b}_{st}")
                        for half in range(2):
                            oneshot.append(outproj_half(
                                b, st, catt, ob, half,
                                act_copy=tail and half == 0))
                    pull_gen(3)

            # batch 0's projections run up front (nothing to hide behind)
            ctx = prepare(0, splits=4)
            gen_box[0] = proj_gen(ctx)
            drain_all()
            for b in range(B):
                nxt = prepare(b + 1) if b + 1 < B else None
                gen_box[0] = proj_gen(nxt) if nxt else None
                attention(ctx)
                drain_all()
                ctx = nxt
    nc.finalize()
    return nc


_NC_CACHE = {}


def _get_nc():
    if "nc" not in _NC_CACHE:
        _NC_CACHE["nc"] = build_nc()
    return _NC_CACHE["nc"]


def kernel(x, Wq, bq, Wk, bk, Wv, bv, Wo, bo):
    x_bf = np.ascontiguousarray(np.asarray(x, dtype=np.float32)).astype(
        ml_dtypes.bfloat16)
    tri = np.triu(np.ones((128, 128), dtype=np.float32)).astype(
        ml_dtypes.bfloat16)
    in_maps = []
    for c in range(NCORES):
        h0, h1 = 2 * c, 2 * c + 1
        m = {
            "x": x_bf,
            "tri": tri,
            "wo": np.ascontiguousarray(Wo[c * KH:(c + 1) * KH]).astype(
                ml_dtypes.bfloat16),
        }
        for nm, W, bb in (("q", Wq, bq), ("k", Wk, bk), ("v", Wv, bv)):
            Wc = np.concatenate([W[h0], W[h1]], axis=1).astype(
                ml_dtypes.bfloat16)
            m["w" + nm] = np.ascontiguousarray(Wc.reshape(DC, 128, KH))
            m["b" + nm] = np.concatenate([bb[h0], bb[h1]]).astype(
                np.float32).reshape(KH, 1)
        in_maps.append(m)
    nc = _get_nc()
    res = run_bass_kernel_spmd(nc, in_maps, list(range(NCORES)))
    acc = np.zeros((B, S, D), dtype=np.float32)
    for c in range(NCORES):
        acc += np.asarray(res.results[c]["out"], dtype=np.float32)
    return acc + np.asarray(bo, dtype=np.float32)[None, None, :]


# revision 42
# speedup vs baseline: 1.6336x; 1.0410x over previous
"""Masked multi-head SDP attention, 8 NeuronCores = (batch, head-half).

Core c owns batch b = c // 2 and heads [half*8, half*8+8) with
half = c % 2. x^T for the core's single batch is loaded once via the XBAR
DMA-transpose and reused by all four head-pair groups. The output
projection accumulates all four groups in PSUM, so each core writes a
single [S, D] bf16 partial (host sums the two halves per batch).

Inner attention structure matches kernel.py (bf16 matmuls, paired-head
score PSUM + 2-region exp, [v0|ones|v1] fused denominator, cross-base
reciprocal normalize, software-pipelined emission with PE fillers).
"""

import sys

sys.path.insert(0, "/opt/trn_rl_repo")

import collections
import numpy as np
import ml_dtypes

import concourse.bass as bass
import concourse.mybir as mybir
from concourse import bacc
from concourse.masks import make_identity
from concourse.tile import TileContext
from concourse.bass_utils import run_bass_kernel_spmd

B, S, D, H = 4, 2048, 1024, 16
DK = D // H  # 64
NCORES = 8
NG = 4  # head-pair groups per core
KH = 2 * DK  # 128 per group
KC = NG * KH  # 512 projection outputs per core
NT = S // 128
NI = S // 512
DC = D // 128

F32 = mybir.dt.float32
BF16 = mybir.dt.bfloat16

VW = 3 * DK  # 192: [v0 | ones | v1] per t-tile in vna


def build_nc():
    nc = bacc.Bacc("TRN2", target_bir_lowering=False, debug=False,
                   num_devices=NCORES)
    x = nc.dram_tensor("x", [S, D], BF16, kind="ExternalInput").ap()
    wq = nc.dram_tensor("wq", [DC, 128, KC], BF16, kind="ExternalInput").ap()
    wk = nc.dram_tensor("wk", [DC, 128, KC], BF16, kind="ExternalInput").ap()
    wv = nc.dram_tensor("wv", [DC, 128, KC], BF16, kind="ExternalInput").ap()
    bq = nc.dram_tensor("bq", [128, NG], F32, kind="ExternalInput").ap()
    bk = nc.dram_tensor("bk", [128, NG], F32, kind="ExternalInput").ap()
    bv = nc.dram_tensor("bv", [128, NG], F32, kind="ExternalInput").ap()
    wo = nc.dram_tensor("wo", [NG, KH, D], BF16, kind="ExternalInput").ap()
    tri = nc.dram_tensor("tri", [128, 128], BF16, kind="ExternalInput").ap()
    out = nc.dram_tensor("out", [S, D], BF16, kind="ExternalOutput").ap()

    with TileContext(nc) as tc:
        with (
            tc.tile_pool(name="const", bufs=1) as cpool,
            tc.tile_pool(name="seq", bufs=2) as qpool,
            tc.tile_pool(name="vn", bufs=2) as vpool,
            tc.tile_pool(name="attn", bufs=1) as apool,
            tc.tile_pool(name="fin", bufs=2) as fpool,
            tc.tile_pool(name="pacc", bufs=2, space="PSUM") as ps_acc,
            tc.tile_pool(name="psc", bufs=2, space="PSUM") as ps_sc,
            tc.tile_pool(name="pv", bufs=1, space="PSUM") as ps_v,
        ):
            # weights + x^T first (first projection group needs them)
            w_sb = {}
            for nm, src in (("q", wq), ("k", wk), ("v", wv)):
                t = cpool.tile([128, DC * KC], BF16, tag="w" + nm,
                               name=f"w_{nm}")
                nc.sync.dma_start(
                    out=t[:].rearrange("p (c k) -> p c k", c=DC),
                    in_=src.rearrange("c p k -> p c k"))
                w_sb[nm] = t

            # x^T: persistent, loaded once (quarter-split, slice-major so
            # the first st-block's chunks land first)
            xts = [cpool.tile([128, S], BF16, tag=f"xt{dc}",
                              name=f"xt_{dc}") for dc in range(DC)]
            for sh in range(4):
                for dc in range(DC):
                    nc.sync.dma_start(
                        out=xts[dc][:, sh * 512:(sh + 1) * 512],
                        in_=x[sh * 512:(sh + 1) * 512,
                              dc * 128:(dc + 1) * 128],
                        transpose=True)

            b_sb = {}
            for nm, src in (("q", bq), ("k", bk), ("v", bv)):
                t = cpool.tile([128, NG], F32, tag="b" + nm, name=f"b_{nm}")
                nc.sync.dma_start(out=t[:], in_=src)
                b_sb[nm] = t
            ident = cpool.tile([128, 128], BF16, tag="ident")
            make_identity(nc, ident[:])
            tri2_sb = cpool.tile([128, 256], BF16, tag="tri")
            nc.sync.dma_start(out=tri2_sb[:, 0:128], in_=tri)
            nc.sync.dma_start(out=tri2_sb[:, 128:256], in_=tri)
            wo_sb = [cpool.tile([KH, D], BF16, tag=f"wo{g}", name=f"wo_{g}")
                     for g in range(NG)]
            for g in range(NG):
                nc.sync.dma_start(out=wo_sb[g][:], in_=wo[g])

            def prepare(g):
                """Per-group tiles (vna ones, qt, kt)."""
                vna = vpool.tile([128, NT * VW], BF16, tag="vna",
                                 name=f"vna_{g}")
                vna_r = vna[:].rearrange("p (j g c) -> p j g c", j=NT, g=3)
                nc.gpsimd.memset(vna_r[:, :, 1:2, :], 1.0)
                qt = qpool.tile([128, S], BF16, tag="qt", name=f"qt_{g}")
                kt = qpool.tile([128, S], BF16, tag="kt", name=f"kt_{g}")
                return dict(g=g, vna=vna, vna_r=vna_r, qt=qt, kt=kt)

            def proj_gen(ctx):
                g = ctx["g"]
                for st in range(NI):
                    sl = slice(st * 512, (st + 1) * 512)
                    for nm in ("q", "k", "v"):
                        acc = ps_acc.tile([128, 512], F32, tag="acc",
                                          name=f"acc_{g}_{st}_{nm}")
                        for dc0 in range(0, DC, 2):
                            for dc in (dc0, dc0 + 1):
                                nc.tensor.matmul(
                                    acc[:],
                                    w_sb[nm][:, dc * KC + g * KH:
                                             dc * KC + (g + 1) * KH],
                                    xts[dc][:, sl], start=(dc == 0),
                                    stop=(dc == DC - 1))
                            yield
                        if nm != "v":
                            dst = ctx["qt"] if nm == "q" else ctx["kt"]
                            nc.vector.tensor_scalar_add(
                                dst[:, sl], acc[:], b_sb[nm][:, g:g + 1])
                        else:
                            vtt = fpool.tile([128, 512], BF16, tag="vtt",
                                             name=f"vtt_{g}_{st}")
                            nc.vector.tensor_scalar_add(
                                vtt[:], acc[:], b_sb["v"][:, g:g + 1])
                            tp = ps_acc.tile([128, 512], F32, tag="acc",
                                             name=f"tp_{g}_{st}")
                            tpb = tp[:].bitcast(BF16)
                            for q in range(4):
                                nc.tensor.transpose(
                                    tpb[:, q * 128:(q + 1) * 128],
                                    vtt[:, q * 128:(q + 1) * 128], ident[:])
                            yield
                            tp_r = tpb[:, 0:512].rearrange(
                                "p (q g c) -> p q g c", q=4, g=2)
                            js = slice(st * 4, (st + 1) * 4)
                            nc.vector.tensor_copy(
                                ctx["vna_r"][:, js, 0:1, :],
                                tp_r[:, :, 0:1, :])
                            nc.vector.tensor_copy(
                                ctx["vna_r"][:, js, 2:3, :],
                                tp_r[:, :, 1:2, :])
                            yield

            oneshot = collections.deque()
            gen_box = [None]

            def pull_gen(n=1):
                for _ in range(n):
                    if gen_box[0] is not None:
                        try:
                            next(gen_box[0])
                            continue
                        except StopIteration:
                            gen_box[0] = None
                    if oneshot:
                        oneshot.popleft()()

            def drain_all():
                while oneshot or gen_box[0] is not None:
                    pull_gen(1)

            def flush_v(vps, vna, item, nj):
                j, off, at = item
                for h in range(2):
                    base = j * VW + h * DK
                    nc.tensor.matmul(
                        vps[h][:, off:512], vna[:, base:base + 2 * DK],
                        at[:, h * 512 + off:h * 512 + 512],
                        start=(j == 0), stop=(j == nj - 1))

            catts = [None] * NG

            def outproj_half(st, half, tail=False):
                """pw[128,512] = sum_g catt_g[:, st] @ wo_g[:, half]."""
                def emit():
                    ob = obs[st]
                    pw = ps_acc.tile([128, 512], F32, tag="acc",
                                     name=f"pw_{st}_{half}")
                    for g in range(NG):
                        nc.tensor.matmul(
                            pw[:], catts[g][:, st * 128:(st + 1) * 128],
                            wo_sb[g][:, half * 512:(half + 1) * 512],
                            start=(g == 0), stop=(g == NG - 1))
                    dst = ob[:, half * 512:(half + 1) * 512]
                    if tail and half == 0:
                        nc.scalar.copy(dst, pw[:])
                    else:
                        nc.vector.tensor_copy(dst, pw[:])
                    if half == 1:
                        nc.sync.dma_start(
                            out=out[st * 128:(st + 1) * 128, :], in_=ob[:])
                return emit

            obs = {}

            def attention(ctx):
                g = ctx["g"]
                qt, kt, vna = ctx["qt"], ctx["kt"], ctx["vna"]
                catt = fpool.tile([128, S], BF16, tag=f"catt{g}",
                                  name=f"catt_{g}")
                catts[g] = catt
                last = g == NG - 1
                for i in range(NI):
                    nj = 4 * i + 4
                    vps = [ps_v.tile([128, 512], F32, tag=f"v{h}",
                                     name=f"vp_{g}_{i}_{h}")
                           for h in range(2)]
                    sq0 = i * 512
                    pend = []
                    for j in range(nj):
                        q = j - 4 * i
                        off = 128 * q if q >= 0 else 0
                        sp = ps_sc.tile([128, 1024], F32, tag="sc",
                                        name=f"sp_{g}_{i}_{j}")
                        for h in range(2):
                            ks = slice(h * DK, (h + 1) * DK)
                            nc.tensor.matmul(
                                sp[:, h * 512 + off:h * 512 + 512],
                                kt[ks, j * 128:(j + 1) * 128],
                                qt[ks, sq0 + off:sq0 + 512],
                                start=True, stop=True)
                        at = apool.tile([128, 1024], BF16, tag=f"at{j}",
                                        name=f"at_{g}_{i}_{j}")
                        sp2 = sp[:].rearrange("p (r c) -> p r c", r=2)
                        at2 = at[:].rearrange("p (r c) -> p r c", r=2)
                        nc.scalar.activation(
                            at2[:, :, off:512], sp2[:, :, off:512],
                            mybir.ActivationFunctionType.Exp, scale=0.125)
                        if q >= 0:
                            nc.gpsimd.tensor_mul(
                                at2[:, :, off:off + 128],
                                at2[:, :, off:off + 128],
                                tri2_sb[:].rearrange("p (r c) -> p r c", r=2))
                        pend.append((j, off, at))
                        if len(pend) > 1:
                            flush_v(vps, vna, pend.pop(0), nj)
                        pull_gen(1)
                    while pend:
                        flush_v(vps, vna, pend.pop(0), nj)

                    rcp = fpool.tile([128, 512], F32, tag="rcp",
                                     name=f"rcp_{g}_{i}")
                    nc.vector.reciprocal(rcp[0:64, :], vps[0][64:128, :])
                    nc.vector.reciprocal(rcp[64:128, :], vps[1][0:64, :])
                    nc.vector.tensor_mul(catt[0:64, sq0:sq0 + 512],
                                         vps[0][0:64, :], rcp[0:64, :])
                    nc.vector.tensor_mul(catt[64:128, sq0:sq0 + 512],
                                         vps[1][64:128, :], rcp[64:128, :])
                    if last:
                        tail = i == NI - 1
                        for st in range(4 * i, 4 * i + 4):
                            obs[st] = fpool.tile([128, D], BF16, tag="ob",
                                                 name=f"ob_{st}")
                            for half in range(2):
                                oneshot.append(
                                    outproj_half(st, half, tail=tail))
                    pull_gen(3)

            ctx = prepare(0)
            gen_box[0] = proj_gen(ctx)
            drain_all()
            for g in range(NG):
                nxt = prepare(g + 1) if g + 1 < NG else None
                gen_box[0] = proj_gen(nxt) if nxt else None
                attention(ctx)
                drain_all()
                ctx = nxt
    nc.finalize()
    return nc


_NC_CACHE = {}


def _get_nc():
    if "nc" not in _NC_CACHE:
        _NC_CACHE["nc"] = build_nc()
    return _NC_CACHE["nc"]


def kernel(x, Wq, bq, Wk, bk, Wv, bv, Wo, bo):
    x_bf = np.ascontiguousarray(np.asarray(x, dtype=np.float32)).astype(
        ml_dtypes.bfloat16)
    tri = np.triu(np.ones((128, 128), dtype=np.float32)).astype(
        ml_dtypes.bfloat16)
    in_maps = []
    for c in range(NCORES):
        b, half = c // 2, c % 2
        hs = [half * 8 + k for k in range(8)]
        m = {"x": x_bf[b], "tri": tri}
        # wo: per group g, rows for heads (2g, 2g+1) of this half
        wo_g = np.stack([
            np.concatenate([Wo[hs[2 * g] * DK:(hs[2 * g] + 1) * DK],
                            Wo[hs[2 * g + 1] * DK:(hs[2 * g + 1] + 1) * DK]],
                           axis=0)
            for g in range(NG)])
        m["wo"] = np.ascontiguousarray(wo_g.astype(ml_dtypes.bfloat16))
        for nm, W, bb in (("q", Wq, bq), ("k", Wk, bk), ("v", Wv, bv)):
            Wc = np.concatenate([W[h] for h in hs], axis=1)  # [D, 512]
            m["w" + nm] = np.ascontiguousarray(
                Wc.reshape(DC, 128, KC).astype(ml_dtypes.bfloat16))
            bc = np.concatenate([bb[h] for h in hs])  # [512]
            m["b" + nm] = np.ascontiguousarray(
                bc.reshape(NG, 128).T.astype(np.float32))
        in_maps.append(m)
    nc = _get_nc()
    res = run_bass_kernel_spmd(nc, in_maps, list(range(NCORES)))
    outp = np.zeros((B, S, D), dtype=np.float32)
    for c in range(NCORES):
        outp[c // 2] += np.asarray(res.results[c]["out"], dtype=np.float32)
    return outp + np.asarray(bo, dtype=np.float32)[None, None, :]


# revision 43
# speedup vs baseline: 1.7508x; 1.0717x over previous
"""Masked multi-head SDP attention, 8 NeuronCores = (batch, head-half).

Core c owns batch b = c // 2 and heads [half*8, half*8+8) with
half = c % 2. x^T for the core's single batch is loaded once via the XBAR
DMA-transpose and reused by all four head-pair groups. The output
projection accumulates all four groups in PSUM, so each core writes a
single [S, D] bf16 partial (host sums the two halves per batch).

Inner attention structure matches kernel.py (bf16 matmuls, paired-head
score PSUM + 2-region exp, [v0|ones|v1] fused denominator, cross-base
reciprocal normalize, software-pipelined emission with PE fillers).
"""

import sys

sys.path.insert(0, "/opt/trn_rl_repo")

import collections
import numpy as np
import ml_dtypes

import concourse.bass as bass
import concourse.mybir as mybir
from concourse import bacc
from concourse.masks import make_identity
from concourse.tile import TileContext
from concourse.bass_utils import run_bass_kernel_spmd

B, S, D, H = 4, 2048, 1024, 16
DK = D // H  # 64
NCORES = 8
NG = 4  # head-pair groups per core
KH = 2 * DK  # 128 per group
KC = NG * KH  # 512 projection outputs per core
NT = S // 128
NI = S // 512
DC = D // 128

F32 = mybir.dt.float32
BF16 = mybir.dt.bfloat16

VW = 3 * DK  # 192: [v0 | ones | v1] per t-tile in vna


def build_nc():
    nc = bacc.Bacc("TRN2", target_bir_lowering=False, debug=False,
                   num_devices=NCORES)
    x = nc.dram_tensor("x", [S, D], BF16, kind="ExternalInput").ap()
    wq = nc.dram_tensor("wq", [DC, 128, KC], BF16, kind="ExternalInput").ap()
    wk = nc.dram_tensor("wk", [DC, 128, KC], BF16, kind="ExternalInput").ap()
    wv = nc.dram_tensor("wv", [DC, 128, KC], BF16, kind="ExternalInput").ap()
    bq = nc.dram_tensor("bq", [128, NG], F32, kind="ExternalInput").ap()
    bk = nc.dram_tensor("bk", [128, NG], F32, kind="ExternalInput").ap()
    bv = nc.dram_tensor("bv", [128, NG], F32, kind="ExternalInput").ap()
    wo = nc.dram_tensor("wo", [NG, KH, D], BF16, kind="ExternalInput").ap()
    tri = nc.dram_tensor("tri", [128, 128], BF16, kind="ExternalInput").ap()
    out = nc.dram_tensor("out", [S, D], BF16, kind="ExternalOutput").ap()

    with TileContext(nc) as tc:
        with (
            tc.tile_pool(name="const", bufs=1) as cpool,
            tc.tile_pool(name="seq", bufs=2) as qpool,
            tc.tile_pool(name="vn", bufs=2) as vpool,
            tc.tile_pool(name="attn", bufs=1) as apool,
            tc.tile_pool(name="fin", bufs=2) as fpool,
            tc.tile_pool(name="pacc", bufs=2, space="PSUM") as ps_acc,
            tc.tile_pool(name="psc", bufs=2, space="PSUM") as ps_sc,
            tc.tile_pool(name="pv", bufs=1, space="PSUM") as ps_v,
        ):
            # x^T quarter 0 first, then weights, then remaining quarters —
            # minimizes time until the first projection group can run
            # (HWDGE issue is serialized at ~625ns/DMA).
            xts = [cpool.tile([128, S], BF16, tag=f"xt{dc}",
                              name=f"xt_{dc}") for dc in range(DC)]

            def xt_quarter(sh):
                for dc in range(DC):
                    nc.sync.dma_start(
                        out=xts[dc][:, sh * 512:(sh + 1) * 512],
                        in_=x[sh * 512:(sh + 1) * 512,
                              dc * 128:(dc + 1) * 128],
                        transpose=True)

            w_sb = {}
            for nm, src in (("q", wq), ("k", wk), ("v", wv)):
                t = cpool.tile([128, DC * KC], BF16, tag="w" + nm,
                               name=f"w_{nm}")
                nc.sync.dma_start(
                    out=t[:].rearrange("p (c k) -> p c k", c=DC),
                    in_=src.rearrange("c p k -> p c k"))
                w_sb[nm] = t
            b_sb = {}
            for nm, src in (("q", bq), ("k", bk), ("v", bv)):
                t = cpool.tile([128, NG], F32, tag="b" + nm, name=f"b_{nm}")
                nc.sync.dma_start(out=t[:], in_=src)
                b_sb[nm] = t
            ident = cpool.tile([128, 128], BF16, tag="ident")
            make_identity(nc, ident[:])
            tri2_sb = cpool.tile([128, 256], BF16, tag="tri")
            nc.sync.dma_start(out=tri2_sb[:, 0:128], in_=tri)
            nc.sync.dma_start(out=tri2_sb[:, 128:256], in_=tri)
            for _sh in range(4):
                xt_quarter(_sh)
            wo_sb = [cpool.tile([KH, D], BF16, tag=f"wo{g}", name=f"wo_{g}")
                     for g in range(NG)]
            for g in range(NG):
                nc.sync.dma_start(out=wo_sb[g][:], in_=wo[g])

            def prepare(g):
                """Per-group tiles (vna ones, qt, kt)."""
                vna = vpool.tile([128, NT * VW], BF16, tag="vna",
                                 name=f"vna_{g}")
                vna_r = vna[:].rearrange("p (j g c) -> p j g c", j=NT, g=3)
                nc.gpsimd.memset(vna_r[:, :, 1:2, :], 1.0)
                qt = qpool.tile([128, S], BF16, tag="qt", name=f"qt_{g}")
                kt = qpool.tile([128, S], BF16, tag="kt", name=f"kt_{g}")
                return dict(g=g, vna=vna, vna_r=vna_r, qt=qt, kt=kt)

            def proj_gen(ctx):
                g = ctx["g"]
                for st in range(NI):
                    sl = slice(st * 512, (st + 1) * 512)
                    for nm in ("q", "k", "v"):
                        acc = ps_acc.tile([128, 512], F32, tag="acc",
                                          name=f"acc_{g}_{st}_{nm}")
                        for dc0 in range(0, DC, 2):
                            for dc in (dc0, dc0 + 1):
                                nc.tensor.matmul(
                                    acc[:],
                                    w_sb[nm][:, dc * KC + g * KH:
                                             dc * KC + (g + 1) * KH],
                                    xts[dc][:, sl], start=(dc == 0),
                                    stop=(dc == DC - 1))
                            yield
                        if nm != "v":
                            dst = ctx["qt"] if nm == "q" else ctx["kt"]
                            nc.vector.tensor_scalar_add(
                                dst[:, sl], acc[:], b_sb[nm][:, g:g + 1])
                        else:
                            vtt = fpool.tile([128, 512], BF16, tag="vtt",
                                             name=f"vtt_{g}_{st}")
                            nc.vector.tensor_scalar_add(
                                vtt[:], acc[:], b_sb["v"][:, g:g + 1])
                            tp = ps_acc.tile([128, 512], F32, tag="acc",
                                             name=f"tp_{g}_{st}")
                            tpb = tp[:].bitcast(BF16)
                            for q in range(4):
                                nc.tensor.transpose(
                                    tpb[:, q * 128:(q + 1) * 128],
                                    vtt[:, q * 128:(q + 1) * 128], ident[:])
                            yield
                            tp_r = tpb[:, 0:512].rearrange(
                                "p (q g c) -> p q g c", q=4, g=2)
                            js = slice(st * 4, (st + 1) * 4)
                            nc.vector.tensor_copy(
                                ctx["vna_r"][:, js, 0:1, :],
                                tp_r[:, :, 0:1, :])
                            nc.vector.tensor_copy(
                                ctx["vna_r"][:, js, 2:3, :],
                                tp_r[:, :, 1:2, :])
                            yield

            oneshot = collections.deque()
            gen_box = [None]

            def pull_gen(n=1):
                for _ in range(n):
                    if gen_box[0] is not None:
                        try:
                            next(gen_box[0])
                            continue
                        except StopIteration:
                            gen_box[0] = None
                    if oneshot:
                        oneshot.popleft()()

            def drain_all():
                while oneshot or gen_box[0] is not None:
                    pull_gen(1)

            def flush_v(vps, vna, item, nj):
                j, off, at = item
                for h in range(2):
                    base = j * VW + h * DK
                    nc.tensor.matmul(
                        vps[h][:, off:512], vna[:, base:base + 2 * DK],
                        at[:, h * 512 + off:h * 512 + 512],
                        start=(j == 0), stop=(j == nj - 1))

            catts = [None] * NG

            def outproj_half(st, half, tail=False):
                """pw[128,512] = sum_g catt_g[:, st] @ wo_g[:, half]."""
                def emit():
                    ob = obs[st]
                    pw = ps_acc.tile([128, 512], F32, tag="acc",
                                     name=f"pw_{st}_{half}")
                    for g in range(NG):
                        nc.tensor.matmul(
                            pw[:], catts[g][:, st * 128:(st + 1) * 128],
                            wo_sb[g][:, half * 512:(half + 1) * 512],
                            start=(g == 0), stop=(g == NG - 1))
                    dst = ob[:, half * 512:(half + 1) * 512]
                    if tail and half == 0:
                        nc.scalar.copy(dst, pw[:])
                    else:
                        nc.vector.tensor_copy(dst, pw[:])
                    if half == 1:
                        nc.sync.dma_start(
                            out=out[st * 128:(st + 1) * 128, :], in_=ob[:])
                return emit

            obs = {}

            def attention(ctx, pump=None):
                g = ctx["g"]
                qt, kt, vna = ctx["qt"], ctx["kt"], ctx["vna"]
                catt = fpool.tile([128, S], BF16, tag=f"catt{g}",
                                  name=f"catt_{g}")
                catts[g] = catt
                last = g == NG - 1
                for i in range(NI):
                    if pump is not None:
                        pump(i)
                    nj = 4 * i + 4
                    vps = [ps_v.tile([128, 512], F32, tag=f"v{h}",
                                     name=f"vp_{g}_{i}_{h}")
                           for h in range(2)]
                    sq0 = i * 512
                    pend = []
                    for j in range(nj):
                        q = j - 4 * i
                        off = 128 * q if q >= 0 else 0
                        sp = ps_sc.tile([128, 1024], F32, tag="sc",
                                        name=f"sp_{g}_{i}_{j}")
                        for h in range(2):
                            ks = slice(h * DK, (h + 1) * DK)
                            nc.tensor.matmul(
                                sp[:, h * 512 + off:h * 512 + 512],
                                kt[ks, j * 128:(j + 1) * 128],
                                qt[ks, sq0 + off:sq0 + 512],
                                start=True, stop=True)
                        at = apool.tile([128, 1024], BF16, tag=f"at{j}",
                                        name=f"at_{g}_{i}_{j}")
                        sp2 = sp[:].rearrange("p (r c) -> p r c", r=2)
                        at2 = at[:].rearrange("p (r c) -> p r c", r=2)
                        nc.scalar.activation(
                            at2[:, :, off:512], sp2[:, :, off:512],
                            mybir.ActivationFunctionType.Exp, scale=0.125)
                        if q >= 0:
                            nc.gpsimd.tensor_mul(
                                at2[:, :, off:off + 128],
                                at2[:, :, off:off + 128],
                                tri2_sb[:].rearrange("p (r c) -> p r c", r=2))
                        pend.append((j, off, at))
                        if len(pend) > 1:
                            flush_v(vps, vna, pend.pop(0), nj)
                        pull_gen(1)
                    while pend:
                        flush_v(vps, vna, pend.pop(0), nj)

                    rcp = fpool.tile([128, 512], F32, tag="rcp",
                                     name=f"rcp_{g}_{i}")
                    nc.vector.reciprocal(rcp[0:64, :], vps[0][64:128, :])
                    nc.vector.reciprocal(rcp[64:128, :], vps[1][0:64, :])
                    nc.vector.tensor_mul(catt[0:64, sq0:sq0 + 512],
                                         vps[0][0:64, :], rcp[0:64, :])
                    nc.vector.tensor_mul(catt[64:128, sq0:sq0 + 512],
                                         vps[1][64:128, :], rcp[64:128, :])
                    if last:
                        tail = i == NI - 1
                        for st in range(4 * i, 4 * i + 4):
                            obs[st] = fpool.tile([128, D], BF16, tag="ob",
                                                 name=f"ob_{st}")
                            for half in range(2):
                                oneshot.append(
                                    outproj_half(st, half, tail=tail))
                    pull_gen(3)

            # All projection groups flow through one chained generator.
            # attention(g) pumps it just far enough that i-block k's inputs
            # (st-groups <= k of group g) are emitted, then keeps pulling it
            # as per-j filler — so PE never waits for late x^T quarters and
            # group g+1's projections interleave into group g's attention.
            YPG = 14  # proj_gen yields per st-group
            progress = [0] * NG
            ctxs = {}

            def chain():
                for gg in range(NG):
                    ctxs[gg] = prepare(gg)
                    for item in proj_gen(ctxs[gg]):
                        progress[gg] += 1
                        yield item

            gen_box[0] = chain()

            def pump(g, i):
                target = (i + 1) * YPG
                while gen_box[0] is not None and progress[g] < target:
                    pull_gen(1)

            for g in range(NG):
                while g not in ctxs and gen_box[0] is not None:
                    pull_gen(1)
                attention(ctxs[g], pump=lambda i, g=g: pump(g, i))
            drain_all()
    nc.finalize()
    return nc


_NC_CACHE = {}


def _get_nc():
    if "nc" not in _NC_CACHE:
        _NC_CACHE["nc"] = build_nc()
    return _NC_CACHE["nc"]


def kernel(x, Wq, bq, Wk, bk, Wv, bv, Wo, bo):
    x_bf = np.ascontiguousarray(np.asarray(x, dtype=np.float32)).astype(
        ml_dtypes.bfloat16)
    tri = np.triu(np.ones((128, 128), dtype=np.float32)).astype(
        ml_dtypes.bfloat16)
    in_maps = []
    for c in range(NCORES):
        b, half = c // 2, c % 2
        hs = [half * 8 + k for k in range(8)]
        m = {"x": x_bf[b], "tri": tri}
        # wo: per group g, rows for heads (2g, 2g+1) of this half
        wo_g = np.stack([
            np.concatenate([Wo[hs[2 * g] * DK:(hs[2 * g] + 1) * DK],
                            Wo[hs[2 * g + 1] * DK:(hs[2 * g + 1] + 1) * DK]],
                           axis=0)
            for g in range(NG)])
        m["wo"] = np.ascontiguousarray(wo_g.astype(ml_dtypes.bfloat16))
        for nm, W, bb in (("q", Wq, bq), ("k", Wk, bk), ("v", Wv, bv)):
            Wc = np.concatenate([W[h] for h in hs], axis=1)  # [D, 512]
            m["w" + nm] = np.ascontiguousarray(
                Wc.reshape(DC, 128, KC).astype(ml_dtypes.bfloat16))
            bc = np.concatenate([bb[h] for h in hs])  # [512]
            m["b" + nm] = np.ascontiguousarray(
                bc.reshape(NG, 128).T.astype(np.float32))
        in_maps.append(m)
    nc = _get_nc()
    res = run_bass_kernel_spmd(nc, in_maps, list(range(NCORES)))
    outp = np.zeros((B, S, D), dtype=np.float32)
    for c in range(NCORES):
        outp[c // 2] += np.asarray(res.results[c]["out"], dtype=np.float32)
    return outp + np.asarray(bo, dtype=np.float32)[None, None, :]


# revision 44
# speedup vs baseline: 1.8603x; 1.0625x over previous
"""Masked multi-head SDP attention, 8 NeuronCores = (batch, head-half).

Core c owns batch b = c // 2 and heads [half*8, half*8+8) with
half = c % 2. x^T for the core's single batch is loaded once via the XBAR
DMA-transpose and reused by all four head-pair groups. The output
projection accumulates all four groups in PSUM, so each core writes a
single [S, D] bf16 partial (host sums the two halves per batch).

Inner attention structure matches kernel.py (bf16 matmuls, paired-head
score PSUM + 2-region exp, [v0|ones|v1] fused denominator, cross-base
reciprocal normalize, software-pipelined emission with PE fillers).
"""

import sys

sys.path.insert(0, "/opt/trn_rl_repo")

import collections
import numpy as np
import ml_dtypes

import concourse.bass as bass
import concourse.mybir as mybir
from concourse import bacc
from concourse.masks import make_identity
from concourse.tile import TileContext
from concourse.bass_utils import run_bass_kernel_spmd

B, S, D, H = 4, 2048, 1024, 16
DK = D // H  # 64
NCORES = 8
NG = 4  # head-pair groups per core
KH = 2 * DK  # 128 per group
KC = NG * KH  # 512 projection outputs per core
NT = S // 128
NI = S // 512
DC = D // 128

F32 = mybir.dt.float32
BF16 = mybir.dt.bfloat16

VW = 3 * DK  # 192: [v0 | ones | v1] per t-tile in vna


def build_nc():
    nc = bacc.Bacc("TRN2", target_bir_lowering=False, debug=False,
                   num_devices=NCORES)
    x = nc.dram_tensor("x", [S, D], BF16, kind="ExternalInput").ap()
    wq = nc.dram_tensor("wq", [DC, 128, KC], BF16, kind="ExternalInput").ap()
    wk = nc.dram_tensor("wk", [DC, 128, KC], BF16, kind="ExternalInput").ap()
    wv = nc.dram_tensor("wv", [DC, 128, KC], BF16, kind="ExternalInput").ap()
    bq = nc.dram_tensor("bq", [128, NG], F32, kind="ExternalInput").ap()
    bk = nc.dram_tensor("bk", [128, NG], F32, kind="ExternalInput").ap()
    bv = nc.dram_tensor("bv", [128, NG], F32, kind="ExternalInput").ap()
    wo = nc.dram_tensor("wo", [NG, KH, D], BF16, kind="ExternalInput").ap()
    tri = nc.dram_tensor("tri", [128, 128], BF16, kind="ExternalInput").ap()
    out = nc.dram_tensor("out", [S, D], BF16, kind="ExternalOutput").ap()

    with TileContext(nc) as tc:
        with (
            tc.tile_pool(name="const", bufs=1) as cpool,
            tc.tile_pool(name="seq", bufs=2) as qpool,
            tc.tile_pool(name="vn", bufs=2) as vpool,
            tc.tile_pool(name="attn", bufs=1) as apool,
            tc.tile_pool(name="fin", bufs=2) as fpool,
            tc.tile_pool(name="pacc", bufs=2, space="PSUM") as ps_acc,
            tc.tile_pool(name="psc", bufs=2, space="PSUM") as ps_sc,
            tc.tile_pool(name="pv", bufs=1, space="PSUM") as ps_v,
        ):
            # x^T quarter 0 first, then weights, then remaining quarters —
            # minimizes time until the first projection group can run
            # (HWDGE issue is serialized at ~625ns/DMA).
            xts = [cpool.tile([128, S], BF16, tag=f"xt{dc}",
                              name=f"xt_{dc}") for dc in range(DC)]

            def xt_quarter(sh):
                for dc in range(DC):
                    nc.sync.dma_start(
                        out=xts[dc][:, sh * 512:(sh + 1) * 512],
                        in_=x[sh * 512:(sh + 1) * 512,
                              dc * 128:(dc + 1) * 128],
                        transpose=True)

            w_sb = {}
            for nm, src in (("q", wq), ("k", wk), ("v", wv)):
                t = cpool.tile([128, DC * KC], BF16, tag="w" + nm,
                               name=f"w_{nm}")
                nc.sync.dma_start(
                    out=t[:].rearrange("p (c k) -> p c k", c=DC),
                    in_=src.rearrange("c p k -> p c k"))
                w_sb[nm] = t
            b_sb = {}
            for nm, src in (("q", bq), ("k", bk), ("v", bv)):
                t = cpool.tile([128, NG], F32, tag="b" + nm, name=f"b_{nm}")
                nc.sync.dma_start(out=t[:], in_=src)
                b_sb[nm] = t
            ident = cpool.tile([128, 128], BF16, tag="ident")
            make_identity(nc, ident[:])
            tri2_sb = cpool.tile([128, 256], BF16, tag="tri")
            nc.sync.dma_start(out=tri2_sb[:, 0:128], in_=tri)
            nc.sync.dma_start(out=tri2_sb[:, 128:256], in_=tri)
            for _sh in range(4):
                xt_quarter(_sh)
            wo_sb = [cpool.tile([KH, D], BF16, tag=f"wo{g}", name=f"wo_{g}")
                     for g in range(NG)]
            for g in range(NG):
                nc.sync.dma_start(out=wo_sb[g][:], in_=wo[g])

            def prepare(g):
                """Per-group tiles (vna ones, qt, kt)."""
                vna = vpool.tile([128, NT * VW], BF16, tag="vna",
                                 name=f"vna_{g}")
                vna_r = vna[:].rearrange("p (j g c) -> p j g c", j=NT, g=3)
                nc.gpsimd.memset(vna_r[:, :, 1:2, :], 1.0)
                qt = qpool.tile([128, S], BF16, tag="qt", name=f"qt_{g}")
                kt = qpool.tile([128, S], BF16, tag="kt", name=f"kt_{g}")
                return dict(g=g, vna=vna, vna_r=vna_r, qt=qt, kt=kt)

            def proj_gen(ctx):
                g = ctx["g"]
                for st in range(NI):
                    sl = slice(st * 512, (st + 1) * 512)
                    for nm in ("q", "k", "v"):
                        acc = ps_acc.tile([128, 512], F32, tag="acc",
                                          name=f"acc_{g}_{st}_{nm}")
                        for dc0 in range(0, DC, 2):
                            for dc in (dc0, dc0 + 1):
                                nc.tensor.matmul(
                                    acc[:],
                                    w_sb[nm][:, dc * KC + g * KH:
                                             dc * KC + (g + 1) * KH],
                                    xts[dc][:, sl], start=(dc == 0),
                                    stop=(dc == DC - 1))
                            yield
                        if nm != "v":
                            dst = ctx["qt"] if nm == "q" else ctx["kt"]
                            nc.vector.tensor_scalar_add(
                                dst[:, sl], acc[:], b_sb[nm][:, g:g + 1])
                        else:
                            vtt = fpool.tile([128, 512], BF16, tag="vtt",
                                             name=f"vtt_{g}_{st}")
                            nc.vector.tensor_scalar_add(
                                vtt[:], acc[:], b_sb["v"][:, g:g + 1])
                            tp = ps_acc.tile([128, 512], F32, tag="acc",
                                             name=f"tp_{g}_{st}")
                            tpb = tp[:].bitcast(BF16)
                            for q in range(4):
                                nc.tensor.transpose(
                                    tpb[:, q * 128:(q + 1) * 128],
                                    vtt[:, q * 128:(q + 1) * 128], ident[:])
                            yield
                            tp_r = tpb[:, 0:512].rearrange(
                                "p (q g c) -> p q g c", q=4, g=2)
                            js = slice(st * 4, (st + 1) * 4)
                            nc.vector.tensor_copy(
                                ctx["vna_r"][:, js, 0:1, :],
                                tp_r[:, :, 0:1, :])
                            nc.vector.tensor_copy(
                                ctx["vna_r"][:, js, 2:3, :],
                                tp_r[:, :, 1:2, :])
                            yield

            oneshot = collections.deque()
            gen_box = [None]

            def pull_gen(n=1):
                for _ in range(n):
                    if gen_box[0] is not None:
                        try:
                            next(gen_box[0])
                            continue
                        except StopIteration:
                            gen_box[0] = None
                    if oneshot:
                        oneshot.popleft()()

            def drain_all():
                while oneshot or gen_box[0] is not None:
                    pull_gen(1)

            def flush_v(vps, vna, item, nj):
                j, off, at = item
                for h in range(2):
                    base = j * VW + h * DK
                    nc.tensor.matmul(
                        vps[h][:, off:512], vna[:, base:base + 2 * DK],
                        at[:, h * 512 + off:h * 512 + 512],
                        start=(j == 0), stop=(j == nj - 1))

            catts = [None] * NG

            def outproj_half(st, half, tail=False):
                """pw[128,512] = sum_g catt_g[:, st] @ wo_g[:, half]."""
                def emit():
                    ob = obs[st]
                    pw = ps_acc.tile([128, 512], F32, tag="acc",
                                     name=f"pw_{st}_{half}")
                    for g in range(NG):
                        nc.tensor.matmul(
                            pw[:], catts[g][:, st * 128:(st + 1) * 128],
                            wo_sb[g][:, half * 512:(half + 1) * 512],
                            start=(g == 0), stop=(g == NG - 1))
                    dst = ob[:, half * 512:(half + 1) * 512]
                    if tail and half == 0:
                        nc.scalar.copy(dst, pw[:])
                    else:
                        nc.vector.tensor_copy(dst, pw[:])
                    if half == 1:
                        nc.sync.dma_start(
                            out=out[st * 128:(st + 1) * 128, :], in_=ob[:])
                return emit

            obs = {}

            def attention(ctx, pump=None):
                g = ctx["g"]
                qt, kt, vna = ctx["qt"], ctx["kt"], ctx["vna"]
                catt = fpool.tile([128, S], BF16, tag=f"catt{g}",
                                  name=f"catt_{g}")
                catts[g] = catt
                last = g == NG - 1
                for i in range(NI):
                    if pump is not None:
                        pump(i)
                    nj = 4 * i + 4
                    vps = [ps_v.tile([128, 512], F32, tag=f"v{h}",
                                     name=f"vp_{g}_{i}_{h}")
                           for h in range(2)]
                    sq0 = i * 512
                    pend = []
                    for j in range(nj):
                        q = j - 4 * i
                        off = 128 * q if q >= 0 else 0
                        sp = ps_sc.tile([128, 1024], F32, tag="sc",
                                        name=f"sp_{g}_{i}_{j}")
                        for h in range(2):
                            ks = slice(h * DK, (h + 1) * DK)
                            nc.tensor.matmul(
                                sp[:, h * 512 + off:h * 512 + 512],
                                kt[ks, j * 128:(j + 1) * 128],
                                qt[ks, sq0 + off:sq0 + 512],
                                start=True, stop=True)
                        at = apool.tile([128, 1024], BF16, tag=f"at{j}",
                                        name=f"at_{g}_{i}_{j}")
                        sp2 = sp[:].rearrange("p (r c) -> p r c", r=2)
                        at2 = at[:].rearrange("p (r c) -> p r c", r=2)
                        nc.scalar.activation(
                            at2[:, :, off:512], sp2[:, :, off:512],
                            mybir.ActivationFunctionType.Exp, scale=0.125)
                        if q >= 0:
                            nc.gpsimd.tensor_mul(
                                at2[:, :, off:off + 128],
                                at2[:, :, off:off + 128],
                                tri2_sb[:].rearrange("p (r c) -> p r c", r=2))
                        pend.append((j, off, at))
                        if len(pend) > (4 if i <= 1 else 3):
                            flush_v(vps, vna, pend.pop(0), nj)
                        pull_gen(1)
                    while pend:
                        flush_v(vps, vna, pend.pop(0), nj)

                    rcp = fpool.tile([128, 512], F32, tag="rcp",
                                     name=f"rcp_{g}_{i}")
                    nc.vector.reciprocal(rcp[0:64, :], vps[0][64:128, :])
                    nc.vector.reciprocal(rcp[64:128, :], vps[1][0:64, :])
                    nc.vector.tensor_mul(catt[0:64, sq0:sq0 + 512],
                                         vps[0][0:64, :], rcp[0:64, :])
                    nc.vector.tensor_mul(catt[64:128, sq0:sq0 + 512],
                                         vps[1][64:128, :], rcp[64:128, :])
                    if last:
                        tail = i == NI - 1
                        for st in range(4 * i, 4 * i + 4):
                            obs[st] = fpool.tile([128, D], BF16, tag="ob",
                                                 name=f"ob_{st}")
                            for half in range(2):
                                oneshot.append(
                                    outproj_half(st, half, tail=tail))
                    pull_gen(4)

            # All projection groups flow through one chained generator.
            # attention(g) pumps it just far enough that i-block k's inputs
            # (st-groups <= k of group g) are emitted, then keeps pulling it
            # as per-j filler — so PE never waits for late x^T quarters and
            # group g+1's projections interleave into group g's attention.
            YPG = 14  # proj_gen yields per st-group
            progress = [0] * NG
            ctxs = {}

            def chain():
                for gg in range(NG):
                    ctxs[gg] = prepare(gg)
                    for item in proj_gen(ctxs[gg]):
                        progress[gg] += 1
                        yield item

            gen_box[0] = chain()

            def pump(g, i):
                target = (i + 1) * YPG
                while gen_box[0] is not None and progress[g] < target:
                    pull_gen(1)

            for g in range(NG):
                while g not in ctxs and gen_box[0] is not None:
                    pull_gen(1)
                attention(ctxs[g], pump=lambda i, g=g: pump(g, i))
            drain_all()
    nc.finalize()
    return nc


_NC_CACHE = {}


def _get_nc():
    if "nc" not in _NC_CACHE:
        _NC_CACHE["nc"] = build_nc()
    return _NC_CACHE["nc"]


def kernel(x, Wq, bq, Wk, bk, Wv, bv, Wo, bo):
    x_bf = np.ascontiguousarray(np.asarray(x, dtype=np.float32)).astype(
        ml_dtypes.bfloat16)
    tri = np.triu(np.ones((128, 128), dtype=np.float32)).astype(
        ml_dtypes.bfloat16)
    in_maps = []
    for c in range(NCORES):
        b, half = c // 2, c % 2
        hs = [half * 8 + k for k in range(8)]
        m = {"x": x_bf[b], "tri": tri}
        # wo: per group g, rows for heads (2g, 2g+1) of this half
        wo_g = np.stack([
            np.concatenate([Wo[hs[2 * g] * DK:(hs[2 * g] + 1) * DK],
                            Wo[hs[2 * g + 1] * DK:(hs[2 * g + 1] + 1) * DK]],
                           axis=0)
            for g in range(NG)])
        m["wo"] = np.ascontiguousarray(wo_g.astype(ml_dtypes.bfloat16))
        for nm, W, bb in (("q", Wq, bq), ("k", Wk, bk), ("v", Wv, bv)):
            Wc = np.concatenate([W[h] for h in hs], axis=1)  # [D, 512]
            m["w" + nm] = np.ascontiguousarray(
                Wc.reshape(DC, 128, KC).astype(ml_dtypes.bfloat16))
            bc = np.concatenate([bb[h] for h in hs])  # [512]
            m["b" + nm] = np.ascontiguousarray(
                bc.reshape(NG, 128).T.astype(np.float32))
        in_maps.append(m)
    nc = _get_nc()
    res = run_bass_kernel_spmd(nc, in_maps, list(range(NCORES)))
    outp = np.zeros((B, S, D), dtype=np.float32)
    for c in range(NCORES):
        outp[c // 2] += np.asarray(res.results[c]["out"], dtype=np.float32)
    return outp + np.asarray(bo, dtype=np.float32)[None, None, :]
